# revision 16
# baseline (speedup 1.0000x reference)
"""LogicLayer Trainium2 kernel v2 — multi-engine split.

out = k0 + k1*a + k2*b + k3*a*b,  k = softmax(w) @ OP_COEFFS (per neuron).

Engine split per chunk (1024 neurons × 512 batch):
  Pool  : SWDGE gathers of a/b rows; softmax coeff-mults + reduces
  DVE   : t1 = k3*a+k2, t2 = k1*a+k0 (per-col tensor_scalar), m = t1*b (TT)
  PE    : psum = I@m + I@t2  (identity-stationary accumulate)
  ACT   : exp(w); psum -> fp16 out tiles (cast during copy)
  HWDGE : input loads + output stores
Layout identical to v1: neuron (p, col) -> partition p, free col; batch on
the innermost free dim; host assembles the transposed fp16 shards.
"""
import numpy as np

from concourse import bacc, mybir, tile, bass
from concourse.bass_utils import run_bass_kernel_spmd

BATCH = 512
IN_DIM = 8192
OUT_DIM = 65536
N_CORES = 8
SHARD = OUT_DIM // N_CORES
P = 128
NCOL = SHARD // P

CH_IDX = 1024
N_CHUNK = SHARD // CH_IDX
COLS_PER_CHUNK = CH_IDX // P
OUT_COLS = COLS_PER_CHUNK
N_BLK = NCOL // OUT_COLS

FP16 = mybir.dt.float16
F32 = mybir.dt.float32
I16 = mybir.dt.int16

OP_COEFFS = np.array([
    [0.0,  0.0,  0.0,  0.0],
    [0.0,  0.0,  0.0,  1.0],
    [0.0,  1.0,  0.0, -1.0],
    [0.0,  1.0,  0.0,  0.0],
    [0.0,  0.0,  1.0, -1.0],
    [0.0,  0.0,  1.0,  0.0],
    [0.0,  1.0,  1.0, -2.0],
    [0.0,  1.0,  1.0, -1.0],
    [1.0, -1.0, -1.0,  1.0],
    [1.0, -1.0, -1.0,  2.0],
    [1.0,  0.0, -1.0,  0.0],
    [1.0,  0.0, -1.0,  1.0],
    [1.0, -1.0,  0.0,  0.0],
    [1.0, -1.0,  0.0,  1.0],
    [1.0,  0.0,  0.0, -1.0],
    [1.0,  0.0,  0.0,  0.0],
], dtype=np.float32)

WORK_BUFS = 3


def build_program(n_reps: int = 1, ch_idx: int = CH_IDX, queues: int = 2,
                  ab_bufs: int = WORK_BUFS, gathers: bool = True,
                  compute: bool = True, out_dma: bool = True,
                  psum_fine: bool = False, kmult_engine: str = 'pool',
                  out_split: bool = True, single_packet: bool = True,
                  gather_elem: int = BATCH):
    n_chunk = SHARD // ch_idx
    cols_per_chunk = ch_idx // P
    nc = bacc.Bacc("TRN2", target_bir_lowering=False, debug=False,
                   num_devices=N_CORES, num_swdge_queues=queues)

    xt = nc.dram_tensor("xt", [IN_DIM, BATCH], FP16, kind="ExternalInput")
    w = nc.dram_tensor("w", [P, NCOL * 16], FP16, kind="ExternalInput")
    coef = nc.dram_tensor("coef", [P, 64], F32, kind="ExternalInput")
    ident = nc.dram_tensor("ident", [P, P], FP16, kind="ExternalInput")
    idxa = nc.dram_tensor("idxa", [P, SHARD // 16], I16, kind="ExternalInput")
    idxb = nc.dram_tensor("idxb", [P, SHARD // 16], I16, kind="ExternalInput")
    out = nc.dram_tensor("out", [P, NCOL, BATCH], FP16, kind="ExternalOutput")

    with tile.TileContext(nc) as tc:
        with tc.tile_pool(name="const", bufs=1) as cpool, \
             tc.tile_pool(name="work", bufs=WORK_BUFS) as pool, \
             tc.tile_pool(name="psum", bufs=(4 if psum_fine else 2),
                          space="PSUM") as ppool:
            ident_sb = cpool.tile([P, P], FP16, tag="ident_sb")
            nc.sync.dma_start(out=ident_sb[:], in_=ident[:])
            coef_sb = cpool.tile([P, 64], F32, tag="coef_sb")
            nc.sync.dma_start(out=coef_sb[:], in_=coef[:])

            for _rep in range(n_reps):
                # ---- k coefficients: k = softmax(w) @ OP_COEFFS
                w_sb = cpool.tile([P, NCOL * 16], FP16, tag="w_sb", bufs=2)
                nc.sync.dma_start(out=w_sb[:], in_=w[:])
                e = cpool.tile([P, NCOL * 16], F32, tag="e", bufs=2)
                nc.scalar.activation(e[:], w_sb[:], mybir.ActivationFunctionType.Exp)
                e3 = e[:].rearrange("p (c i) -> p c i", i=16)

                s = cpool.tile([P, NCOL], F32, tag="s", bufs=2)
                nc.vector.tensor_reduce(out=s[:], in_=e3, axis=mybir.AxisListType.X,
                                        op=mybir.AluOpType.add)
                rs = cpool.tile([P, NCOL], F32, tag="rs", bufs=2)
                nc.vector.reciprocal(rs[:], s[:])

                k = []
                for cc in range(4):
                    m4 = cpool.tile([P, NCOL * 16], F32, tag=f"ktmp{cc}", bufs=2)
                    cb = coef_sb[:, cc * 16:(cc + 1) * 16].unsqueeze(1).broadcast_to(
                        [P, NCOL, 16])
                    keng = nc.gpsimd if kmult_engine == 'pool' else nc.vector
                    keng.tensor_tensor(
                        out=m4[:].rearrange("p (c i) -> p c i", i=16),
                        in0=e3, in1=cb, op=mybir.AluOpType.mult)
                    ks = cpool.tile([P, NCOL], F32, tag=f"ksum{cc}", bufs=2)
                    nc.vector.tensor_reduce(
                        out=ks[:], in_=m4[:].rearrange("p (c i) -> p c i", i=16),
                        axis=mybir.AxisListType.X, op=mybir.AluOpType.add)
                    kc = cpool.tile([P, NCOL], F32, tag=f"k{cc}", bufs=2)
                    nc.vector.tensor_tensor(out=kc[:], in0=ks[:], in1=rs[:],
                                            op=mybir.AluOpType.mult)
                    k.append(kc)

                # ---- main loop
                ia_all = cpool.tile([P, SHARD // 16], I16, tag="ia_all", bufs=2)
                ib_all = cpool.tile([P, SHARD // 16], I16, tag="ib_all", bufs=2)
                nc.sync.dma_start(out=ia_all[:], in_=idxa[:])
                nc.sync.dma_start(out=ib_all[:], in_=idxb[:])
                if not gathers and _rep == 0:
                    a_fix = cpool.tile([P, cols_per_chunk, BATCH], FP16, tag="a_fix")
                    b_fix = cpool.tile([P, cols_per_chunk, BATCH], FP16, tag="b_fix")
                    nc.vector.memset(a_fix[:], 0.25)
                    nc.vector.memset(b_fix[:], 0.25)
                for chunk in range(n_chunk):
                    csl = slice(chunk * (ch_idx // 16), (chunk + 1) * (ch_idx // 16))
                    if gathers:
                        ge = gather_elem
                        a_t = pool.tile([P, cols_per_chunk, ge], FP16, tag="a",
                                        bufs=ab_bufs)
                        b_t = pool.tile([P, cols_per_chunk, ge], FP16, tag="b",
                                        bufs=ab_bufs)
                        nc.gpsimd.dma_gather(
                            out_ap=a_t[:], in_ap=xt[:, :ge],
                            idxs_ap=ia_all[:, csl],
                            num_idxs=ch_idx, num_idxs_reg=ch_idx, elem_size=ge,
                            elem_step=BATCH,
                            queue_num=(2 * chunk) % queues,
                            single_packet=single_packet)
                        nc.gpsimd.dma_gather(
                            out_ap=b_t[:], in_ap=xt[:, :ge],
                            idxs_ap=ib_all[:, csl],
                            num_idxs=ch_idx, num_idxs_reg=ch_idx, elem_size=ge,
                            elem_step=BATCH,
                            queue_num=(2 * chunk + 1) % queues,
                            single_packet=single_packet)
                    else:
                        a_t, b_t = a_fix, b_fix
                    if not compute:
                        if out_dma and gathers:
                            nc.sync.dma_start(
                                out=out[:, chunk * cols_per_chunk:(chunk + 1) * cols_per_chunk, :],
                                in_=a_t[:])
                        continue

                    t1 = pool.tile([P, cols_per_chunk, BATCH], FP16, tag="t1")
                    t2 = pool.tile([P, cols_per_chunk, BATCH], FP16, tag="t2")
                    for g in range(cols_per_chunk):
                        col = chunk * cols_per_chunk + g
                        a_sl = a_t[:, g, :]
                        nc.vector.tensor_scalar(
                            out=t1[:, g, :], in0=a_sl,
                            scalar1=k[3][:, col:col + 1],
                            scalar2=k[2][:, col:col + 1],
                            op0=mybir.AluOpType.mult, op1=mybir.AluOpType.add)
                        nc.vector.tensor_scalar(
                            out=t2[:, g, :], in0=a_sl,
                            scalar1=k[1][:, col:col + 1],
                            scalar2=k[0][:, col:col + 1],
                            op0=mybir.AluOpType.mult, op1=mybir.AluOpType.add)
                    nc.vector.tensor_tensor(out=t1[:], in0=t1[:], in1=b_t[:],
                                            op=mybir.AluOpType.mult)
                    m = t1

                    out_t = pool.tile([P, cols_per_chunk, BATCH], FP16, tag="out_t")
                    half_cols = max(1, cols_per_chunk // (4 if psum_fine else 2))
                    for h in range(cols_per_chunk // half_cols):
                        psum = ppool.tile([P, half_cols, BATCH], F32, tag="ps")
                        for g2 in range(half_cols):
                            g = h * half_cols + g2
                            nc.tensor.matmul(psum[:, g2, :], ident_sb[:],
                                             m[:, g, :], start=True, stop=False)
                            nc.tensor.matmul(psum[:, g2, :], ident_sb[:],
                                             t2[:, g, :], start=False, stop=True)
                        hs = slice(h * half_cols, (h + 1) * half_cols)
                        nc.scalar.activation(out_t[:, hs, :], psum[:],
                                             mybir.ActivationFunctionType.Copy)
                        if out_dma and out_split:
                            c0 = chunk * cols_per_chunk + h * half_cols
                            nc.sync.dma_start(
                                out=out[:, c0:c0 + half_cols, :],
                                in_=out_t[:, hs, :])
                    if out_dma and out_split:
                        pass  # emitted per half below
                    elif out_dma:
                        nc.sync.dma_start(
                            out=out[:, chunk * cols_per_chunk:(chunk + 1) * cols_per_chunk, :],
                            in_=out_t[:])
    nc.compile()
    return nc


def make_in_maps(x, weights, connections, ch_idx=CH_IDX):
    n_chunk = SHARD // ch_idx
    xt = np.ascontiguousarray(x.T.astype(np.float16))
    coef_dev = np.tile(OP_COEFFS.T.reshape(1, 64), (P, 1)).astype(np.float32)
    coef_dev = np.ascontiguousarray(coef_dev)
    ident = np.eye(P, dtype=np.float16)

    in_maps = []
    for c in range(N_CORES):
        base = c * SHARD
        w_shard = weights[base:base + SHARD]
        w_dev = np.ascontiguousarray(
            w_shard.reshape(NCOL, P, 16).transpose(1, 0, 2).reshape(P, NCOL * 16)
        ).astype(np.float16)

        conn = connections[base:base + SHARD].astype(np.int16)
        idx = []
        for j in range(2):
            arr = conn[:, j].reshape(n_chunk, ch_idx // 16, 16).transpose(0, 2, 1)
            arr = np.tile(arr, (1, 8, 1))
            idx.append(np.ascontiguousarray(
                arr.transpose(1, 0, 2).reshape(P, -1)))
        in_maps.append({
            "xt": xt, "w": w_dev, "coef": coef_dev, "ident": ident,
            "idxa": idx[0], "idxb": idx[1],
        })
    return in_maps


def assemble_output(results):
    shards = []
    for c in range(N_CORES):
        o = results[c]["out"]  # [P, NCOL, BATCH]; neuron col*128+p at [p, col]
        shards.append(o.transpose(1, 0, 2).reshape(SHARD, BATCH))
    full = np.concatenate(shards, axis=0)
    return np.ascontiguousarray(full.T.astype(np.float32))


FP8 = mybir.dt.float8e4


def build_program_fp8(n_reps: int = 1, queues: int = 4, t2_act_cols: int = 3,
                      ab_bufs: int = 3, work_bufs: int = 3,
                      gathers: bool = True, compute: bool = True,
                      out_dma: bool = True):
    """fp8-gather variant: x gathered as e4m3 (512B descs), PE identity-matmul
    casts fp8->psum f32; ACT computes t1=k3*a+k2 (and part of t2) during the
    psum->SBUF traversal; DVE does the rest of t2, m=t1*b, out=m+t2."""
    ch_idx = CH_IDX
    n_chunk = SHARD // ch_idx
    cols = ch_idx // P  # 8 cols per chunk
    nc = bacc.Bacc("TRN2", target_bir_lowering=False, debug=False,
                   num_devices=N_CORES, num_swdge_queues=queues)

    xt = nc.dram_tensor("xt", [IN_DIM, BATCH], FP8, kind="ExternalInput")
    w = nc.dram_tensor("w", [P, NCOL * 16], FP16, kind="ExternalInput")
    coef = nc.dram_tensor("coef", [P, 64], F32, kind="ExternalInput")
    ident = nc.dram_tensor("ident", [P, P], FP8, kind="ExternalInput")
    idxa = nc.dram_tensor("idxa", [P, SHARD // 16], I16, kind="ExternalInput")
    idxb = nc.dram_tensor("idxb", [P, SHARD // 16], I16, kind="ExternalInput")
    out = nc.dram_tensor("out", [P, NCOL, BATCH], FP16, kind="ExternalOutput")

    with tile.TileContext(nc) as tc:
        with tc.tile_pool(name="const", bufs=1) as cpool, \
             tc.tile_pool(name="work", bufs=work_bufs) as pool, \
             tc.tile_pool(name="psum", bufs=2, space="PSUM") as ppool:
            ident_sb = cpool.tile([P, P], FP8, tag="ident_sb")
            nc.sync.dma_start(out=ident_sb[:], in_=ident[:])
            coef_sb = cpool.tile([P, 64], F32, tag="coef_sb")
            nc.sync.dma_start(out=coef_sb[:], in_=coef[:])

            for _rep in range(n_reps):
                # ---- k coefficients: k = softmax(w) @ OP_COEFFS
                w_sb = cpool.tile([P, NCOL * 16], FP16, tag="w_sb", bufs=2)
                nc.sync.dma_start(out=w_sb[:], in_=w[:])
                e = cpool.tile([P, NCOL * 16], F32, tag="e", bufs=2)
                nc.scalar.activation(e[:], w_sb[:], mybir.ActivationFunctionType.Exp)
                e3 = e[:].rearrange("p (c i) -> p c i", i=16)

                s = cpool.tile([P, NCOL], F32, tag="s", bufs=2)
                nc.vector.tensor_reduce(out=s[:], in_=e3, axis=mybir.AxisListType.X,
                                        op=mybir.AluOpType.add)
                rs = cpool.tile([P, NCOL], F32, tag="rs", bufs=2)
                nc.vector.reciprocal(rs[:], s[:])

                k = []
                for cc in range(4):
                    m4 = cpool.tile([P, NCOL * 16], F32, tag=f"ktmp{cc}", bufs=2)
                    cb = coef_sb[:, cc * 16:(cc + 1) * 16].unsqueeze(1).broadcast_to(
                        [P, NCOL, 16])
                    nc.gpsimd.tensor_tensor(
                        out=m4[:].rearrange("p (c i) -> p c i", i=16),
                        in0=e3, in1=cb, op=mybir.AluOpType.mult)
                    ks = cpool.tile([P, NCOL], F32, tag=f"ksum{cc}", bufs=2)
                    nc.vector.tensor_reduce(
                        out=ks[:], in_=m4[:].rearrange("p (c i) -> p c i", i=16),
                        axis=mybir.AxisListType.X, op=mybir.AluOpType.add)
                    kc = cpool.tile([P, NCOL], F32, tag=f"k{cc}", bufs=2)
                    nc.vector.tensor_tensor(out=kc[:], in0=ks[:], in1=rs[:],
                                            op=mybir.AluOpType.mult)
                    k.append(kc)

                ia_all = cpool.tile([P, SHARD // 16], I16, tag="ia_all", bufs=2)
                ib_all = cpool.tile([P, SHARD // 16], I16, tag="ib_all", bufs=2)
                nc.sync.dma_start(out=ia_all[:], in_=idxa[:])
                nc.sync.dma_start(out=ib_all[:], in_=idxb[:])
                if not gathers and _rep == 0:
                    a8_fix = cpool.tile([P, cols, BATCH], FP8, tag="a8_fix")
                    b8_fix = cpool.tile([P, cols, BATCH], FP8, tag="b8_fix")
                    nc.vector.memset(a8_fix[:], 0.25)
                    nc.vector.memset(b8_fix[:], 0.25)

                for chunk in range(n_chunk):
                    csl = slice(chunk * (ch_idx // 16), (chunk + 1) * (ch_idx // 16))
                    if gathers:
                        a8 = pool.tile([P, cols, BATCH], FP8, tag="a8",
                                       bufs=ab_bufs)
                        b8 = pool.tile([P, cols, BATCH], FP8, tag="b8",
                                       bufs=ab_bufs)
                    else:
                        a8, b8 = a8_fix, b8_fix
                    if gathers:
                        nc.gpsimd.dma_gather(
                            out_ap=a8[:], in_ap=xt[:], idxs_ap=ia_all[:, csl],
                            num_idxs=ch_idx, num_idxs_reg=ch_idx, elem_size=BATCH,
                            queue_num=(2 * chunk) % queues)
                        nc.gpsimd.dma_gather(
                            out_ap=b8[:], in_ap=xt[:], idxs_ap=ib_all[:, csl],
                            num_idxs=ch_idx, num_idxs_reg=ch_idx, elem_size=BATCH,
                            queue_num=(2 * chunk + 1) % queues)
                    if not compute:
                        if out_dma and gathers:
                            nc.sync.dma_start(
                                out=out[:, chunk * cols:(chunk + 1) * cols, :256],
                                in_=a8[:].bitcast(FP16))
                        continue

                    t1 = pool.tile([P, cols, BATCH], FP16, tag="t1")
                    t2 = pool.tile([P, cols, BATCH], FP16, tag="t2")
                    b16 = pool.tile([P, cols, BATCH], FP16, tag="b16")
                    for q in range(cols // 2):
                        psa = ppool.tile([P, 2, BATCH], F32, tag="psa")
                        psb = ppool.tile([P, 2, BATCH], F32, tag="psb")
                        for j in range(2):
                            lc = 2 * q + j
                            nc.tensor.matmul(psa[:, j, :], ident_sb[:],
                                             a8[:, lc, :], start=True, stop=True)
                            nc.tensor.matmul(psb[:, j, :], ident_sb[:],
                                             b8[:, lc, :], start=True, stop=True)
                        nc.scalar.activation(b16[:, 2 * q:2 * q + 2, :], psb[:],
                                             mybir.ActivationFunctionType.Copy)
                        for j in range(2):
                            lc = 2 * q + j
                            col = chunk * cols + lc
                            nc.scalar.activation(
                                t1[:, lc, :], psa[:, j, :],
                                mybir.ActivationFunctionType.Identity,
                                scale=k[3][:, col:col + 1],
                                bias=k[2][:, col:col + 1])
                            if lc < t2_act_cols:
                                nc.scalar.activation(
                                    t2[:, lc, :], psa[:, j, :],
                                    mybir.ActivationFunctionType.Identity,
                                    scale=k[1][:, col:col + 1],
                                    bias=k[0][:, col:col + 1])
                            else:
                                nc.vector.tensor_scalar(
                                    out=t2[:, lc, :], in0=psa[:, j, :],
                                    scalar1=k[1][:, col:col + 1],
                                    scalar2=k[0][:, col:col + 1],
                                    op0=mybir.AluOpType.mult,
                                    op1=mybir.AluOpType.add)
                    nc.vector.tensor_tensor(out=t1[:], in0=t1[:], in1=b16[:],
                                            op=mybir.AluOpType.mult)
                    out_t = pool.tile([P, cols, BATCH], FP16, tag="out_t")
                    nc.vector.tensor_tensor(out=out_t[:], in0=t1[:], in1=t2[:],
                                            op=mybir.AluOpType.add)
                    if out_dma:
                        nc.sync.dma_start(
                            out=out[:, chunk * cols:(chunk + 1) * cols, :],
                            in_=out_t[:])
    nc.compile()
    return nc


def make_in_maps_fp8(x, weights, connections, ch_idx=CH_IDX):
    n_chunk = SHARD // ch_idx
    xt8 = np.ascontiguousarray(x.T).astype(mybir.dt.np(FP8))
    coef_dev = np.tile(OP_COEFFS.T.reshape(1, 64), (P, 1)).astype(np.float32)
    coef_dev = np.ascontiguousarray(coef_dev)
    ident = np.eye(P).astype(mybir.dt.np(FP8))

    in_maps = []
    for c in range(N_CORES):
        base = c * SHARD
        w_shard = weights[base:base + SHARD]
        w_dev = np.ascontiguousarray(
            w_shard.reshape(NCOL, P, 16).transpose(1, 0, 2).reshape(P, NCOL * 16)
        ).astype(np.float16)

        conn = connections[base:base + SHARD].astype(np.int16)
        idx = []
        for j in range(2):
            arr = conn[:, j].reshape(n_chunk, ch_idx // 16, 16).transpose(0, 2, 1)
            arr = np.tile(arr, (1, 8, 1))
            idx.append(np.ascontiguousarray(
                arr.transpose(1, 0, 2).reshape(P, -1)))
        in_maps.append({
            "xt": xt8, "w": w_dev, "coef": coef_dev, "ident": ident,
            "idxa": idx[0], "idxb": idx[1],
        })
    return in_maps


def build_program_e(n_reps: int = 1, queues: int = 4,
                    t1_eng: str = 'act', t2_eng: str = 'act',
                    bcast_eng: str = 'dve',
                    ab_bufs: int = 3, work_bufs: int = 3,
                    gathers: bool = True, compute: bool = True,
                    out_dma: bool = True):
    """Mixed-precision gather variant: a gathered fp16 (1KB descs), b gathered
    fp8 e4m3 (512B descs).  No PE/PSUM: b8 cast to fp16 by a DVE/ACT copy;
    t1 = k3*a+k2 and t2 = k1*a+k0 from fp16 a; m = t1*b16; out = m + t2."""
    ch_idx = CH_IDX
    n_chunk = SHARD // ch_idx
    cols = ch_idx // P
    nc = bacc.Bacc("TRN2", target_bir_lowering=False, debug=False,
                   num_devices=N_CORES, num_swdge_queues=queues)

    xta = nc.dram_tensor("xta", [IN_DIM, BATCH], FP16, kind="ExternalInput")
    xtb = nc.dram_tensor("xtb", [IN_DIM, BATCH], FP8, kind="ExternalInput")
    w = nc.dram_tensor("w", [P, NCOL * 16], FP16, kind="ExternalInput")
    coef = nc.dram_tensor("coef", [P, 64], F32, kind="ExternalInput")
    idxa = nc.dram_tensor("idxa", [P, SHARD // 16], I16, kind="ExternalInput")
    idxb = nc.dram_tensor("idxb", [P, SHARD // 16], I16, kind="ExternalInput")
    out = nc.dram_tensor("out", [P, NCOL, BATCH], FP16, kind="ExternalOutput")

    with tile.TileContext(nc) as tc:
        with tc.tile_pool(name="const", bufs=1) as cpool, \
             tc.tile_pool(name="work", bufs=work_bufs) as pool:
            coef_sb = cpool.tile([P, 64], F32, tag="coef_sb")
            nc.sync.dma_start(out=coef_sb[:], in_=coef[:])

            for _rep in range(n_reps):
                # ---- k coefficients: k = softmax(w) @ OP_COEFFS
                w_sb = cpool.tile([P, NCOL * 16], FP16, tag="w_sb", bufs=2)
                nc.sync.dma_start(out=w_sb[:], in_=w[:])
                e = cpool.tile([P, NCOL * 16], F32, tag="e", bufs=2)
                nc.scalar.activation(e[:], w_sb[:], mybir.ActivationFunctionType.Exp)
                e3 = e[:].rearrange("p (c i) -> p c i", i=16)

                s = cpool.tile([P, NCOL], F32, tag="s", bufs=2)
                nc.vector.tensor_reduce(out=s[:], in_=e3, axis=mybir.AxisListType.X,
                                        op=mybir.AluOpType.add)
                rs = cpool.tile([P, NCOL], F32, tag="rs", bufs=2)
                nc.vector.reciprocal(rs[:], s[:])

                k = []
                for cc in range(4):
                    m4 = cpool.tile([P, NCOL * 16], F32, tag=f"ktmp{cc}", bufs=2)
                    cb = coef_sb[:, cc * 16:(cc + 1) * 16].unsqueeze(1).broadcast_to(
                        [P, NCOL, 16])
                    nc.gpsimd.tensor_tensor(
                        out=m4[:].rearrange("p (c i) -> p c i", i=16),
                        in0=e3, in1=cb, op=mybir.AluOpType.mult)
                    ks = cpool.tile([P, NCOL], F32, tag=f"ksum{cc}", bufs=2)
                    nc.vector.tensor_reduce(
                        out=ks[:], in_=m4[:].rearrange("p (c i) -> p c i", i=16),
                        axis=mybir.AxisListType.X, op=mybir.AluOpType.add)
                    kc = cpool.tile([P, NCOL], F32, tag=f"k{cc}", bufs=2)
                    nc.vector.tensor_tensor(out=kc[:], in0=ks[:], in1=rs[:],
                                            op=mybir.AluOpType.mult)
                    k.append(kc)

                ia_all = cpool.tile([P, SHARD // 16], I16, tag="ia_all", bufs=2)
                ib_all = cpool.tile([P, SHARD // 16], I16, tag="ib_all", bufs=2)
                nc.sync.dma_start(out=ia_all[:], in_=idxa[:])
                nc.sync.dma_start(out=ib_all[:], in_=idxb[:])
                if not gathers and _rep == 0:
                    a_fix = cpool.tile([P, cols, BATCH], FP16, tag="a_fix")
                    b8_fix = cpool.tile([P, cols, BATCH], FP8, tag="b8_fix")
                    nc.vector.memset(a_fix[:], 0.25)
                    nc.vector.memset(b8_fix[:], 0.25)

                for chunk in range(n_chunk):
                    csl = slice(chunk * (ch_idx // 16), (chunk + 1) * (ch_idx // 16))
                    if gathers:
                        a16 = pool.tile([P, cols, BATCH], FP16, tag="a16",
                                        bufs=ab_bufs)
                        b8 = pool.tile([P, cols, BATCH], FP8, tag="b8",
                                       bufs=ab_bufs)
                        nc.gpsimd.dma_gather(
                            out_ap=a16[:], in_ap=xta[:], idxs_ap=ia_all[:, csl],
                            num_idxs=ch_idx, num_idxs_reg=ch_idx, elem_size=BATCH,
                            queue_num=(2 * chunk) % queues)
                        nc.gpsimd.dma_gather(
                            out_ap=b8[:], in_ap=xtb[:], idxs_ap=ib_all[:, csl],
                            num_idxs=ch_idx, num_idxs_reg=ch_idx, elem_size=BATCH,
                            queue_num=(2 * chunk + 1) % queues)
                    else:
                        a16, b8 = a_fix, b8_fix
                    if not compute:
                        if out_dma and gathers:
                            nc.sync.dma_start(
                                out=out[:, chunk * cols:(chunk + 1) * cols, :],
                                in_=a16[:])
                        continue

                    b16 = pool.tile([P, cols, BATCH], FP16, tag="b16")
                    if bcast_eng == 'dve':
                        nc.vector.tensor_copy(out=b16[:], in_=b8[:])
                    else:
                        nc.scalar.activation(b16[:], b8[:],
                                             mybir.ActivationFunctionType.Copy)
                    t1 = pool.tile([P, cols, BATCH], FP16, tag="t1")
                    t2 = pool.tile([P, cols, BATCH], FP16, tag="t2")
                    for g in range(cols):
                        col = chunk * cols + g
                        for dst, khi, klo, eng in ((t1, 3, 2, t1_eng),
                                                   (t2, 1, 0, t2_eng)):
                            if eng == 'act':
                                nc.scalar.activation(
                                    dst[:, g, :], a16[:, g, :],
                                    mybir.ActivationFunctionType.Identity,
                                    scale=k[khi][:, col:col + 1],
                                    bias=k[klo][:, col:col + 1])
                            else:
                                nc.vector.tensor_scalar(
                                    out=dst[:, g, :], in0=a16[:, g, :],
                                    scalar1=k[khi][:, col:col + 1],
                                    scalar2=k[klo][:, col:col + 1],
                                    op0=mybir.AluOpType.mult,
                                    op1=mybir.AluOpType.add)
                    nc.vector.tensor_tensor(out=t1[:], in0=t1[:], in1=b16[:],
                                            op=mybir.AluOpType.mult)
                    out_t = pool.tile([P, cols, BATCH], FP16, tag="out_t")
                    nc.vector.tensor_tensor(out=out_t[:], in0=t1[:], in1=t2[:],
                                            op=mybir.AluOpType.add)
                    if out_dma:
                        nc.sync.dma_start(
                            out=out[:, chunk * cols:(chunk + 1) * cols, :],
                            in_=out_t[:])
    nc.compile()
    return nc


def make_in_maps_e(x, weights, connections, ch_idx=CH_IDX):
    n_chunk = SHARD // ch_idx
    xt = np.ascontiguousarray(x.T)
    xta = xt.astype(np.float16)
    xtb = xt.astype(mybir.dt.np(FP8))
    coef_dev = np.tile(OP_COEFFS.T.reshape(1, 64), (P, 1)).astype(np.float32)
    coef_dev = np.ascontiguousarray(coef_dev)

    in_maps = []
    for c in range(N_CORES):
        base = c * SHARD
        w_shard = weights[base:base + SHARD]
        w_dev = np.ascontiguousarray(
            w_shard.reshape(NCOL, P, 16).transpose(1, 0, 2).reshape(P, NCOL * 16)
        ).astype(np.float16)

        conn = connections[base:base + SHARD].astype(np.int16)
        idx = []
        for j in range(2):
            arr = conn[:, j].reshape(n_chunk, ch_idx // 16, 16).transpose(0, 2, 1)
            arr = np.tile(arr, (1, 8, 1))
            idx.append(np.ascontiguousarray(
                arr.transpose(1, 0, 2).reshape(P, -1)))
        in_maps.append({
            "xta": xta, "xtb": xtb, "w": w_dev, "coef": coef_dev,
            "idxa": idx[0], "idxb": idx[1],
        })
    return in_maps


def build_program_f2(n_reps: int = 1, queues: int = 4, t2_act_cols: int = 4,
                     ab_bufs: int = 3, work_bufs: int = 3,
                     kmult_engine: str = 'dve',
                     gathers: bool = True, compute: bool = True,
                     out_dma: bool = True):
    """Both-fp8 gathers; PE identity-casts a8/b8 into PSUM quarters; ACT
    evacuates psum->fp16 (plain Copy); DVE does t1/t2 (tensor_scalar 4x on
    fp16) with t2_act_cols of t2 moved to ACT Identity, then m=t1*b16 and
    out=m+t2; one HWDGE store per chunk."""
    ch_idx = CH_IDX
    n_chunk = SHARD // ch_idx
    cols = ch_idx // P
    nc = bacc.Bacc("TRN2", target_bir_lowering=False, debug=False,
                   num_devices=N_CORES, num_swdge_queues=queues)

    xt = nc.dram_tensor("xt", [IN_DIM, BATCH], FP8, kind="ExternalInput")
    w = nc.dram_tensor("w", [P, NCOL * 16], FP16, kind="ExternalInput")
    coef = nc.dram_tensor("coef", [P, 64], F32, kind="ExternalInput")
    ident = nc.dram_tensor("ident", [P, P], FP8, kind="ExternalInput")
    idxa = nc.dram_tensor("idxa", [P, SHARD // 16], I16, kind="ExternalInput")
    idxb = nc.dram_tensor("idxb", [P, SHARD // 16], I16, kind="ExternalInput")
    out = nc.dram_tensor("out", [P, NCOL, BATCH], FP16, kind="ExternalOutput")

    with tile.TileContext(nc) as tc:
        with tc.tile_pool(name="const", bufs=1) as cpool, \
             tc.tile_pool(name="work", bufs=work_bufs) as pool, \
             tc.tile_pool(name="psum", bufs=2, space="PSUM") as ppool:
            ident_sb = cpool.tile([P, P], FP8, tag="ident_sb")
            nc.sync.dma_start(out=ident_sb[:], in_=ident[:])
            coef_sb = cpool.tile([P, 64], F32, tag="coef_sb")
            nc.sync.dma_start(out=coef_sb[:], in_=coef[:])

            for _rep in range(n_reps):
                # ---- k coefficients: k = softmax(w) @ OP_COEFFS
                w_sb = cpool.tile([P, NCOL * 16], FP16, tag="w_sb", bufs=2)
                nc.sync.dma_start(out=w_sb[:], in_=w[:])
                e = cpool.tile([P, NCOL * 16], F32, tag="e", bufs=2)
                nc.scalar.activation(e[:], w_sb[:], mybir.ActivationFunctionType.Exp)
                e3 = e[:].rearrange("p (c i) -> p c i", i=16)

                s = cpool.tile([P, NCOL], F32, tag="s", bufs=2)
                nc.vector.tensor_reduce(out=s[:], in_=e3, axis=mybir.AxisListType.X,
                                        op=mybir.AluOpType.add)
                rs = cpool.tile([P, NCOL], F32, tag="rs", bufs=2)
                nc.vector.reciprocal(rs[:], s[:])

                k = []
                for cc in range(4):
                    m4 = cpool.tile([P, NCOL * 16], F32, tag=f"ktmp{cc}", bufs=2)
                    cb = coef_sb[:, cc * 16:(cc + 1) * 16].unsqueeze(1).broadcast_to(
                        [P, NCOL, 16])
                    keng = nc.gpsimd if kmult_engine == 'pool' else nc.vector
                    keng.tensor_tensor(
                        out=m4[:].rearrange("p (c i) -> p c i", i=16),
                        in0=e3, in1=cb, op=mybir.AluOpType.mult)
                    ks = cpool.tile([P, NCOL], F32, tag=f"ksum{cc}", bufs=2)
                    nc.vector.tensor_reduce(
                        out=ks[:], in_=m4[:].rearrange("p (c i) -> p c i", i=16),
                        axis=mybir.AxisListType.X, op=mybir.AluOpType.add)
                    kc = cpool.tile([P, NCOL], F32, tag=f"k{cc}", bufs=2)
                    nc.vector.tensor_tensor(out=kc[:], in0=ks[:], in1=rs[:],
                                            op=mybir.AluOpType.mult)
                    k.append(kc)

                ia_all = cpool.tile([P, SHARD // 16], I16, tag="ia_all", bufs=2)
                ib_all = cpool.tile([P, SHARD // 16], I16, tag="ib_all", bufs=2)
                nc.sync.dma_start(out=ia_all[:], in_=idxa[:])
                nc.sync.dma_start(out=ib_all[:], in_=idxb[:])
                if not gathers and _rep == 0:
                    a8_fix = cpool.tile([P, cols, BATCH], FP8, tag="a8_fix")
                    b8_fix = cpool.tile([P, cols, BATCH], FP8, tag="b8_fix")
                    nc.vector.memset(a8_fix[:], 0.25)
                    nc.vector.memset(b8_fix[:], 0.25)

                for chunk in range(n_chunk):
                    csl = slice(chunk * (ch_idx // 16), (chunk + 1) * (ch_idx // 16))
                    if gathers:
                        a8 = pool.tile([P, cols, BATCH], FP8, tag="a8",
                                       bufs=ab_bufs)
                        b8 = pool.tile([P, cols, BATCH], FP8, tag="b8",
                                       bufs=ab_bufs)
                        nc.gpsimd.dma_gather(
                            out_ap=a8[:], in_ap=xt[:], idxs_ap=ia_all[:, csl],
                            num_idxs=ch_idx, num_idxs_reg=ch_idx, elem_size=BATCH,
                            queue_num=(2 * chunk) % queues)
                        nc.gpsimd.dma_gather(
                            out_ap=b8[:], in_ap=xt[:], idxs_ap=ib_all[:, csl],
                            num_idxs=ch_idx, num_idxs_reg=ch_idx, elem_size=BATCH,
                            queue_num=(2 * chunk + 1) % queues)
                    else:
                        a8, b8 = a8_fix, b8_fix
                    if not compute:
                        if out_dma and gathers:
                            nc.sync.dma_start(
                                out=out[:, chunk * cols:(chunk + 1) * cols, :256],
                                in_=a8[:].bitcast(FP16))
                        continue

                    a16 = pool.tile([P, cols, BATCH], FP16, tag="a16")
                    b16 = pool.tile([P, cols, BATCH], FP16, tag="b16")
                    for q in range(cols // 2):
                        psa = ppool.tile([P, 2, BATCH], F32, tag="psa")
                        psb = ppool.tile([P, 2, BATCH], F32, tag="psb")
                        for j in range(2):
                            lc = 2 * q + j
                            nc.tensor.matmul(psa[:, j, :], ident_sb[:],
                                             a8[:, lc, :], start=True, stop=True)
                            nc.tensor.matmul(psb[:, j, :], ident_sb[:],
                                             b8[:, lc, :], start=True, stop=True)
                        nc.scalar.activation(a16[:, 2 * q:2 * q + 2, :], psa[:],
                                             mybir.ActivationFunctionType.Copy)
                        nc.scalar.activation(b16[:, 2 * q:2 * q + 2, :], psb[:],
                                             mybir.ActivationFunctionType.Copy)
                    t1 = pool.tile([P, cols, BATCH], FP16, tag="t1")
                    t2 = pool.tile([P, cols, BATCH], FP16, tag="t2")
                    for g in range(cols):
                        col = chunk * cols + g
                        nc.vector.tensor_scalar(
                            out=t1[:, g, :], in0=a16[:, g, :],
                            scalar1=k[3][:, col:col + 1],
                            scalar2=k[2][:, col:col + 1],
                            op0=mybir.AluOpType.mult, op1=mybir.AluOpType.add)
                        if g < t2_act_cols:
                            nc.scalar.activation(
                                t2[:, g, :], a16[:, g, :],
                                mybir.ActivationFunctionType.Identity,
                                scale=k[1][:, col:col + 1],
                                bias=k[0][:, col:col + 1])
                        else:
                            nc.vector.tensor_scalar(
                                out=t2[:, g, :], in0=a16[:, g, :],
                                scalar1=k[1][:, col:col + 1],
                                scalar2=k[0][:, col:col + 1],
                                op0=mybir.AluOpType.mult,
                                op1=mybir.AluOpType.add)
                    nc.vector.tensor_tensor(out=t1[:], in0=t1[:], in1=b16[:],
                                            op=mybir.AluOpType.mult)
                    out_t = pool.tile([P, cols, BATCH], FP16, tag="out_t")
                    nc.vector.tensor_tensor(out=out_t[:], in0=t1[:], in1=t2[:],
                                            op=mybir.AluOpType.add)
                    if out_dma:
                        nc.sync.dma_start(
                            out=out[:, chunk * cols:(chunk + 1) * cols, :],
                            in_=out_t[:])
    nc.compile()
    return nc


def build_program_f2k(n_reps: int = 1, queues: int = 4, t2_act_cols: int = 8,
                      ab_bufs: int = 3, work_bufs: int = 3,
                      kmult_engine: str = 'dve', store_halves: bool = False,
                      gathers: bool = True, compute: bool = True,
                      out_dma: bool = True):
    """f2 (t2-on-ACT) with the k-coefficient phase software-pipelined one rep
    ahead: rep N's chunk loop consumes k computed during rep N-1, so the
    exp/reduce/mult chain never sits on the critical path between gather
    batches."""
    ch_idx = CH_IDX
    n_chunk = SHARD // ch_idx
    cols = ch_idx // P
    nc = bacc.Bacc("TRN2", target_bir_lowering=False, debug=False,
                   num_devices=N_CORES, num_swdge_queues=queues)

    xt = nc.dram_tensor("xt", [IN_DIM, BATCH], FP8, kind="ExternalInput")
    w = nc.dram_tensor("w", [P, NCOL * 16], FP16, kind="ExternalInput")
    coef = nc.dram_tensor("coef", [P, 64], F32, kind="ExternalInput")
    ident = nc.dram_tensor("ident", [P, P], FP8, kind="ExternalInput")
    idxa = nc.dram_tensor("idxa", [P, SHARD // 16], I16, kind="ExternalInput")
    idxb = nc.dram_tensor("idxb", [P, SHARD // 16], I16, kind="ExternalInput")
    out = nc.dram_tensor("out", [P, NCOL, BATCH], FP16, kind="ExternalOutput")

    with tile.TileContext(nc) as tc:
        with tc.tile_pool(name="const", bufs=1) as cpool, \
             tc.tile_pool(name="work", bufs=work_bufs) as pool, \
             tc.tile_pool(name="psum", bufs=2, space="PSUM") as ppool:
            ident_sb = cpool.tile([P, P], FP8, tag="ident_sb")
            nc.sync.dma_start(out=ident_sb[:], in_=ident[:])
            coef_sb = cpool.tile([P, 64], F32, tag="coef_sb")
            nc.sync.dma_start(out=coef_sb[:], in_=coef[:])

            def k_phase():
                w_sb = cpool.tile([P, NCOL * 16], FP16, tag="w_sb", bufs=2)
                nc.sync.dma_start(out=w_sb[:], in_=w[:])
                e = cpool.tile([P, NCOL * 16], F32, tag="e", bufs=2)
                nc.scalar.activation(e[:], w_sb[:],
                                     mybir.ActivationFunctionType.Exp)
                e3 = e[:].rearrange("p (c i) -> p c i", i=16)
                s = cpool.tile([P, NCOL], F32, tag="s", bufs=2)
                nc.vector.tensor_reduce(out=s[:], in_=e3,
                                        axis=mybir.AxisListType.X,
                                        op=mybir.AluOpType.add)
                rs = cpool.tile([P, NCOL], F32, tag="rs", bufs=2)
                nc.vector.reciprocal(rs[:], s[:])
                k = []
                for cc in range(4):
                    m4 = cpool.tile([P, NCOL * 16], F32, tag=f"ktmp{cc}", bufs=2)
                    cb = coef_sb[:, cc * 16:(cc + 1) * 16].unsqueeze(1)
                    cb = cb.broadcast_to([P, NCOL, 16])
                    keng = nc.gpsimd if kmult_engine == 'pool' else nc.vector
                    keng.tensor_tensor(
                        out=m4[:].rearrange("p (c i) -> p c i", i=16),
                        in0=e3, in1=cb, op=mybir.AluOpType.mult)
                    ks = cpool.tile([P, NCOL], F32, tag=f"ksum{cc}", bufs=2)
                    nc.vector.tensor_reduce(
                        out=ks[:], in_=m4[:].rearrange("p (c i) -> p c i", i=16),
                        axis=mybir.AxisListType.X, op=mybir.AluOpType.add)
                    kc = cpool.tile([P, NCOL], F32, tag=f"k{cc}", bufs=2)
                    nc.vector.tensor_tensor(out=kc[:], in0=ks[:], in1=rs[:],
                                            op=mybir.AluOpType.mult)
                    k.append(kc)
                return k

            k = k_phase()  # prologue: k for the first rep
            for _rep in range(n_reps):
                ia_all = cpool.tile([P, SHARD // 16], I16, tag="ia_all", bufs=2)
                ib_all = cpool.tile([P, SHARD // 16], I16, tag="ib_all", bufs=2)
                nc.sync.dma_start(out=ia_all[:], in_=idxa[:])
                nc.sync.dma_start(out=ib_all[:], in_=idxb[:])
                if not gathers and _rep == 0:
                    a8_fix = cpool.tile([P, cols, BATCH], FP8, tag="a8_fix")
                    b8_fix = cpool.tile([P, cols, BATCH], FP8, tag="b8_fix")
                    nc.vector.memset(a8_fix[:], 0.25)
                    nc.vector.memset(b8_fix[:], 0.25)

                for chunk in range(n_chunk):
                    csl = slice(chunk * (ch_idx // 16), (chunk + 1) * (ch_idx // 16))
                    if gathers:
                        a8 = pool.tile([P, cols, BATCH], FP8, tag="a8",
                                       bufs=ab_bufs)
                        b8 = pool.tile([P, cols, BATCH], FP8, tag="b8",
                                       bufs=ab_bufs)
                        nc.gpsimd.dma_gather(
                            out_ap=a8[:], in_ap=xt[:], idxs_ap=ia_all[:, csl],
                            num_idxs=ch_idx, num_idxs_reg=ch_idx, elem_size=BATCH,
                            queue_num=(2 * chunk) % queues)
                        nc.gpsimd.dma_gather(
                            out_ap=b8[:], in_ap=xt[:], idxs_ap=ib_all[:, csl],
                            num_idxs=ch_idx, num_idxs_reg=ch_idx, elem_size=BATCH,
                            queue_num=(2 * chunk + 1) % queues)
                    else:
                        a8, b8 = a8_fix, b8_fix
                    if not compute:
                        if out_dma and gathers:
                            nc.sync.dma_start(
                                out=out[:, chunk * cols:(chunk + 1) * cols, :256],
                                in_=a8[:].bitcast(FP16))
                        continue

                    a16 = pool.tile([P, cols, BATCH], FP16, tag="a16")
                    b16 = pool.tile([P, cols, BATCH], FP16, tag="b16")
                    for q in range(cols // 2):
                        psa = ppool.tile([P, 2, BATCH], F32, tag="psa")
                        psb = ppool.tile([P, 2, BATCH], F32, tag="psb")
                        for j in range(2):
                            lc = 2 * q + j
                            nc.tensor.matmul(psa[:, j, :], ident_sb[:],
                                             a8[:, lc, :], start=True, stop=True)
                            nc.tensor.matmul(psb[:, j, :], ident_sb[:],
                                             b8[:, lc, :], start=True, stop=True)
                        nc.scalar.activation(a16[:, 2 * q:2 * q + 2, :], psa[:],
                                             mybir.ActivationFunctionType.Copy)
                        nc.scalar.activation(b16[:, 2 * q:2 * q + 2, :], psb[:],
                                             mybir.ActivationFunctionType.Copy)
                    t1 = pool.tile([P, cols, BATCH], FP16, tag="t1")
                    t2 = pool.tile([P, cols, BATCH], FP16, tag="t2")
                    for g in range(cols):
                        col = chunk * cols + g
                        nc.vector.tensor_scalar(
                            out=t1[:, g, :], in0=a16[:, g, :],
                            scalar1=k[3][:, col:col + 1],
                            scalar2=k[2][:, col:col + 1],
                            op0=mybir.AluOpType.mult, op1=mybir.AluOpType.add)
                        if g < t2_act_cols:
                            nc.scalar.activation(
                                t2[:, g, :], a16[:, g, :],
                                mybir.ActivationFunctionType.Identity,
                                scale=k[1][:, col:col + 1],
                                bias=k[0][:, col:col + 1])
                        else:
                            nc.vector.tensor_scalar(
                                out=t2[:, g, :], in0=a16[:, g, :],
                                scalar1=k[1][:, col:col + 1],
                                scalar2=k[0][:, col:col + 1],
                                op0=mybir.AluOpType.mult,
                                op1=mybir.AluOpType.add)
                    out_t = pool.tile([P, cols, BATCH], FP16, tag="out_t")
                    if store_halves:
                        for h in range(2):
                            hs = slice(4 * h, 4 * h + 4)
                            nc.vector.tensor_tensor(
                                out=t1[:, hs, :], in0=t1[:, hs, :],
                                in1=b16[:, hs, :], op=mybir.AluOpType.mult)
                            nc.vector.tensor_tensor(
                                out=out_t[:, hs, :], in0=t1[:, hs, :],
                                in1=t2[:, hs, :], op=mybir.AluOpType.add)
                            if out_dma:
                                c0 = chunk * cols + 4 * h
                                nc.sync.dma_start(out=out[:, c0:c0 + 4, :],
                                                  in_=out_t[:, hs, :])
                    else:
                        nc.vector.tensor_tensor(out=t1[:], in0=t1[:],
                                                in1=b16[:],
                                                op=mybir.AluOpType.mult)
                        nc.vector.tensor_tensor(out=out_t[:], in0=t1[:],
                                                in1=t2[:],
                                                op=mybir.AluOpType.add)
                        if out_dma:
                            nc.sync.dma_start(
                                out=out[:, chunk * cols:(chunk + 1) * cols, :],
                                in_=out_t[:])
                if compute and _rep < n_reps - 1:
                    k = k_phase()  # k for the next rep, off the critical path
    nc.compile()
    return nc


def build_program_f2b(n_reps: int = 1, queues: int = 4, t2_act_cols: int = 8,
                      ab_bufs: int = 3, work_bufs: int = 3,
                      kmult_engine: str = 'dve', store_halves: bool = True,
                      gathers: bool = True, compute: bool = True,
                      out_dma: bool = True):
    """f2 with quarter-interleaved emission: per 2-col quarter do
    PE casts -> ACT evac -> t1/t2 -> DVE m -> DVE add, store per half-chunk.
    Shorter dependency chains let chunks pipeline tighter."""
    ch_idx = CH_IDX
    n_chunk = SHARD // ch_idx
    cols = ch_idx // P
    nc = bacc.Bacc("TRN2", target_bir_lowering=False, debug=False,
                   num_devices=N_CORES, num_swdge_queues=queues)

    xt = nc.dram_tensor("xt", [IN_DIM, BATCH], FP8, kind="ExternalInput")
    w = nc.dram_tensor("w", [P, NCOL * 16], FP16, kind="ExternalInput")
    coef = nc.dram_tensor("coef", [P, 64], F32, kind="ExternalInput")
    ident = nc.dram_tensor("ident", [P, P], FP8, kind="ExternalInput")
    idxa = nc.dram_tensor("idxa", [P, SHARD // 16], I16, kind="ExternalInput")
    idxb = nc.dram_tensor("idxb", [P, SHARD // 16], I16, kind="ExternalInput")
    out = nc.dram_tensor("out", [P, NCOL, BATCH], FP16, kind="ExternalOutput")

    with tile.TileContext(nc) as tc:
        with tc.tile_pool(name="const", bufs=1) as cpool, \
             tc.tile_pool(name="work", bufs=work_bufs) as pool, \
             tc.tile_pool(name="psum", bufs=2, space="PSUM") as ppool:
            ident_sb = cpool.tile([P, P], FP8, tag="ident_sb")
            nc.sync.dma_start(out=ident_sb[:], in_=ident[:])
            coef_sb = cpool.tile([P, 64], F32, tag="coef_sb")
            nc.sync.dma_start(out=coef_sb[:], in_=coef[:])

            for _rep in range(n_reps):
                w_sb = cpool.tile([P, NCOL * 16], FP16, tag="w_sb", bufs=2)
                nc.sync.dma_start(out=w_sb[:], in_=w[:])
                e = cpool.tile([P, NCOL * 16], F32, tag="e", bufs=2)
                nc.scalar.activation(e[:], w_sb[:], mybir.ActivationFunctionType.Exp)
                e3 = e[:].rearrange("p (c i) -> p c i", i=16)

                s = cpool.tile([P, NCOL], F32, tag="s", bufs=2)
                nc.vector.tensor_reduce(out=s[:], in_=e3, axis=mybir.AxisListType.X,
                                        op=mybir.AluOpType.add)
                rs = cpool.tile([P, NCOL], F32, tag="rs", bufs=2)
                nc.vector.reciprocal(rs[:], s[:])

                k = []
                for cc in range(4):
                    m4 = cpool.tile([P, NCOL * 16], F32, tag=f"ktmp{cc}", bufs=2)
                    cb = coef_sb[:, cc * 16:(cc + 1) * 16].unsqueeze(1).broadcast_to(
                        [P, NCOL, 16])
                    keng = nc.gpsimd if kmult_engine == 'pool' else nc.vector
                    keng.tensor_tensor(
                        out=m4[:].rearrange("p (c i) -> p c i", i=16),
                        in0=e3, in1=cb, op=mybir.AluOpType.mult)
                    ks = cpool.tile([P, NCOL], F32, tag=f"ksum{cc}", bufs=2)
                    nc.vector.tensor_reduce(
                        out=ks[:], in_=m4[:].rearrange("p (c i) -> p c i", i=16),
                        axis=mybir.AxisListType.X, op=mybir.AluOpType.add)
                    kc = cpool.tile([P, NCOL], F32, tag=f"k{cc}", bufs=2)
                    nc.vector.tensor_tensor(out=kc[:], in0=ks[:], in1=rs[:],
                                            op=mybir.AluOpType.mult)
                    k.append(kc)

                ia_all = cpool.tile([P, SHARD // 16], I16, tag="ia_all", bufs=2)
                ib_all = cpool.tile([P, SHARD // 16], I16, tag="ib_all", bufs=2)
                nc.sync.dma_start(out=ia_all[:], in_=idxa[:])
                nc.sync.dma_start(out=ib_all[:], in_=idxb[:])
                if not gathers and _rep == 0:
                    a8_fix = cpool.tile([P, cols, BATCH], FP8, tag="a8_fix")
                    b8_fix = cpool.tile([P, cols, BATCH], FP8, tag="b8_fix")
                    nc.vector.memset(a8_fix[:], 0.25)
                    nc.vector.memset(b8_fix[:], 0.25)

                for chunk in range(n_chunk):
                    csl = slice(chunk * (ch_idx // 16), (chunk + 1) * (ch_idx // 16))
                    if gathers:
                        a8 = pool.tile([P, cols, BATCH], FP8, tag="a8",
                                       bufs=ab_bufs)
                        b8 = pool.tile([P, cols, BATCH], FP8, tag="b8",
                                       bufs=ab_bufs)
                        nc.gpsimd.dma_gather(
                            out_ap=a8[:], in_ap=xt[:], idxs_ap=ia_all[:, csl],
                            num_idxs=ch_idx, num_idxs_reg=ch_idx, elem_size=BATCH,
                            queue_num=(2 * chunk) % queues)
                        nc.gpsimd.dma_gather(
                            out_ap=b8[:], in_ap=xt[:], idxs_ap=ib_all[:, csl],
                            num_idxs=ch_idx, num_idxs_reg=ch_idx, elem_size=BATCH,
                            queue_num=(2 * chunk + 1) % queues)
                    else:
                        a8, b8 = a8_fix, b8_fix
                    if not compute:
                        if out_dma and gathers:
                            nc.sync.dma_start(
                                out=out[:, chunk * cols:(chunk + 1) * cols, :256],
                                in_=a8[:].bitcast(FP16))
                        continue

                    a16 = pool.tile([P, cols, BATCH], FP16, tag="a16")
                    b16 = pool.tile([P, cols, BATCH], FP16, tag="b16")
                    t1 = pool.tile([P, cols, BATCH], FP16, tag="t1")
                    t2 = pool.tile([P, cols, BATCH], FP16, tag="t2")
                    out_t = pool.tile([P, cols, BATCH], FP16, tag="out_t")
                    for q in range(cols // 2):
                        qs = slice(2 * q, 2 * q + 2)
                        psa = ppool.tile([P, 2, BATCH], F32, tag="psa")
                        psb = ppool.tile([P, 2, BATCH], F32, tag="psb")
                        for j in range(2):
                            lc = 2 * q + j
                            nc.tensor.matmul(psa[:, j, :], ident_sb[:],
                                             a8[:, lc, :], start=True, stop=True)
                            nc.tensor.matmul(psb[:, j, :], ident_sb[:],
                                             b8[:, lc, :], start=True, stop=True)
                        nc.scalar.activation(a16[:, qs, :], psa[:],
                                             mybir.ActivationFunctionType.Copy)
                        nc.scalar.activation(b16[:, qs, :], psb[:],
                                             mybir.ActivationFunctionType.Copy)
                        for j in range(2):
                            lc = 2 * q + j
                            col = chunk * cols + lc
                            nc.vector.tensor_scalar(
                                out=t1[:, lc, :], in0=a16[:, lc, :],
                                scalar1=k[3][:, col:col + 1],
                                scalar2=k[2][:, col:col + 1],
                                op0=mybir.AluOpType.mult,
                                op1=mybir.AluOpType.add)
                            if lc < t2_act_cols:
                                nc.scalar.activation(
                                    t2[:, lc, :], a16[:, lc, :],
                                    mybir.ActivationFunctionType.Identity,
                                    scale=k[1][:, col:col + 1],
                                    bias=k[0][:, col:col + 1])
                            else:
                                nc.vector.tensor_scalar(
                                    out=t2[:, lc, :], in0=a16[:, lc, :],
                                    scalar1=k[1][:, col:col + 1],
                                    scalar2=k[0][:, col:col + 1],
                                    op0=mybir.AluOpType.mult,
                                    op1=mybir.AluOpType.add)
                        nc.vector.tensor_tensor(
                            out=t1[:, qs, :], in0=t1[:, qs, :],
                            in1=b16[:, qs, :], op=mybir.AluOpType.mult)
                        nc.vector.tensor_tensor(
                            out=out_t[:, qs, :], in0=t1[:, qs, :],
                            in1=t2[:, qs, :], op=mybir.AluOpType.add)
                        if out_dma and store_halves and q % 2 == 1:
                            hs = slice(2 * q - 2, 2 * q + 2)
                            c0 = chunk * cols + 2 * q - 2
                            nc.sync.dma_start(out=out[:, c0:c0 + 4, :],
                                              in_=out_t[:, hs, :])
                    if out_dma and not store_halves:
                        nc.sync.dma_start(
                            out=out[:, chunk * cols:(chunk + 1) * cols, :],
                            in_=out_t[:])
    nc.compile()
    return nc


_CACHED_NC = None


def kernel(x, weights, connections):
    global _CACHED_NC
    if _CACHED_NC is None:
        _CACHED_NC = build_program_f2k(t2_act_cols=6)
    in_maps = make_in_maps_fp8(np.asarray(x), np.asarray(weights),
                               np.asarray(connections))
    last_err = None
    for _attempt in range(3):
        try:
            res = run_bass_kernel_spmd(_CACHED_NC, in_maps, list(range(N_CORES)))
            return assemble_output(res.results)
        except Exception as e:
            last_err = e
    raise last_err



# revision 17
# speedup vs baseline: 1.2158x; 1.2158x over previous
"""LogicLayer Trainium2 kernel v2 — multi-engine split.

out = k0 + k1*a + k2*b + k3*a*b,  k = softmax(w) @ OP_COEFFS (per neuron).

Engine split per chunk (1024 neurons × 512 batch):
  Pool  : SWDGE gathers of a/b rows; softmax coeff-mults + reduces
  DVE   : t1 = k3*a+k2, t2 = k1*a+k0 (per-col tensor_scalar), m = t1*b (TT)
  PE    : psum = I@m + I@t2  (identity-stationary accumulate)
  ACT   : exp(w); psum -> fp16 out tiles (cast during copy)
  HWDGE : input loads + output stores
Layout identical to v1: neuron (p, col) -> partition p, free col; batch on
the innermost free dim; host assembles the transposed fp16 shards.
"""
import numpy as np

from concourse import bacc, mybir, tile, bass
from concourse.bass_utils import run_bass_kernel_spmd

BATCH = 512
IN_DIM = 8192
OUT_DIM = 65536
N_CORES = 8
SHARD = OUT_DIM // N_CORES
P = 128
NCOL = SHARD // P

CH_IDX = 1024
N_CHUNK = SHARD // CH_IDX
COLS_PER_CHUNK = CH_IDX // P
OUT_COLS = COLS_PER_CHUNK
N_BLK = NCOL // OUT_COLS

FP16 = mybir.dt.float16
F32 = mybir.dt.float32
I16 = mybir.dt.int16

OP_COEFFS = np.array([
    [0.0,  0.0,  0.0,  0.0],
    [0.0,  0.0,  0.0,  1.0],
    [0.0,  1.0,  0.0, -1.0],
    [0.0,  1.0,  0.0,  0.0],
    [0.0,  0.0,  1.0, -1.0],
    [0.0,  0.0,  1.0,  0.0],
    [0.0,  1.0,  1.0, -2.0],
    [0.0,  1.0,  1.0, -1.0],
    [1.0, -1.0, -1.0,  1.0],
    [1.0, -1.0, -1.0,  2.0],
    [1.0,  0.0, -1.0,  0.0],
    [1.0,  0.0, -1.0,  1.0],
    [1.0, -1.0,  0.0,  0.0],
    [1.0, -1.0,  0.0,  1.0],
    [1.0,  0.0,  0.0, -1.0],
    [1.0,  0.0,  0.0,  0.0],
], dtype=np.float32)

WORK_BUFS = 3


def build_program(n_reps: int = 1, ch_idx: int = CH_IDX, queues: int = 2,
                  ab_bufs: int = WORK_BUFS, gathers: bool = True,
                  compute: bool = True, out_dma: bool = True,
                  psum_fine: bool = False, kmult_engine: str = 'pool',
                  out_split: bool = True, single_packet: bool = True,
                  gather_elem: int = BATCH):
    n_chunk = SHARD // ch_idx
    cols_per_chunk = ch_idx // P
    nc = bacc.Bacc("TRN2", target_bir_lowering=False, debug=False,
                   num_devices=N_CORES, num_swdge_queues=queues)

    xt = nc.dram_tensor("xt", [IN_DIM, BATCH], FP16, kind="ExternalInput")
    w = nc.dram_tensor("w", [P, NCOL * 16], FP16, kind="ExternalInput")
    coef = nc.dram_tensor("coef", [P, 64], F32, kind="ExternalInput")
    ident = nc.dram_tensor("ident", [P, P], FP16, kind="ExternalInput")
    idxa = nc.dram_tensor("idxa", [P, SHARD // 16], I16, kind="ExternalInput")
    idxb = nc.dram_tensor("idxb", [P, SHARD // 16], I16, kind="ExternalInput")
    out = nc.dram_tensor("out", [P, NCOL, BATCH], FP16, kind="ExternalOutput")

    with tile.TileContext(nc) as tc:
        with tc.tile_pool(name="const", bufs=1) as cpool, \
             tc.tile_pool(name="work", bufs=WORK_BUFS) as pool, \
             tc.tile_pool(name="psum", bufs=(4 if psum_fine else 2),
                          space="PSUM") as ppool:
            ident_sb = cpool.tile([P, P], FP16, tag="ident_sb")
            nc.sync.dma_start(out=ident_sb[:], in_=ident[:])
            coef_sb = cpool.tile([P, 64], F32, tag="coef_sb")
            nc.sync.dma_start(out=coef_sb[:], in_=coef[:])

            for _rep in range(n_reps):
                # ---- k coefficients: k = softmax(w) @ OP_COEFFS
                w_sb = cpool.tile([P, NCOL * 16], FP16, tag="w_sb", bufs=2)
                nc.sync.dma_start(out=w_sb[:], in_=w[:])
                e = cpool.tile([P, NCOL * 16], F32, tag="e", bufs=2)
                nc.scalar.activation(e[:], w_sb[:], mybir.ActivationFunctionType.Exp)
                e3 = e[:].rearrange("p (c i) -> p c i", i=16)

                s = cpool.tile([P, NCOL], F32, tag="s", bufs=2)
                nc.vector.tensor_reduce(out=s[:], in_=e3, axis=mybir.AxisListType.X,
                                        op=mybir.AluOpType.add)
                rs = cpool.tile([P, NCOL], F32, tag="rs", bufs=2)
                nc.vector.reciprocal(rs[:], s[:])

                k = []
                for cc in range(4):
                    m4 = cpool.tile([P, NCOL * 16], F32, tag=f"ktmp{cc}", bufs=2)
                    cb = coef_sb[:, cc * 16:(cc + 1) * 16].unsqueeze(1).broadcast_to(
                        [P, NCOL, 16])
                    keng = nc.gpsimd if kmult_engine == 'pool' else nc.vector
                    keng.tensor_tensor(
                        out=m4[:].rearrange("p (c i) -> p c i", i=16),
                        in0=e3, in1=cb, op=mybir.AluOpType.mult)
                    ks = cpool.tile([P, NCOL], F32, tag=f"ksum{cc}", bufs=2)
                    nc.vector.tensor_reduce(
                        out=ks[:], in_=m4[:].rearrange("p (c i) -> p c i", i=16),
                        axis=mybir.AxisListType.X, op=mybir.AluOpType.add)
                    kc = cpool.tile([P, NCOL], F32, tag=f"k{cc}", bufs=2)
                    nc.vector.tensor_tensor(out=kc[:], in0=ks[:], in1=rs[:],
                                            op=mybir.AluOpType.mult)
                    k.append(kc)

                # ---- main loop
                ia_all = cpool.tile([P, SHARD // 16], I16, tag="ia_all", bufs=2)
                ib_all = cpool.tile([P, SHARD // 16], I16, tag="ib_all", bufs=2)
                nc.sync.dma_start(out=ia_all[:], in_=idxa[:])
                nc.sync.dma_start(out=ib_all[:], in_=idxb[:])
                if not gathers and _rep == 0:
                    a_fix = cpool.tile([P, cols_per_chunk, BATCH], FP16, tag="a_fix")
                    b_fix = cpool.tile([P, cols_per_chunk, BATCH], FP16, tag="b_fix")
                    nc.vector.memset(a_fix[:], 0.25)
                    nc.vector.memset(b_fix[:], 0.25)
                for chunk in range(n_chunk):
                    csl = slice(chunk * (ch_idx // 16), (chunk + 1) * (ch_idx // 16))
                    if gathers:
                        ge = gather_elem
                        a_t = pool.tile([P, cols_per_chunk, ge], FP16, tag="a",
                                        bufs=ab_bufs)
                        b_t = pool.tile([P, cols_per_chunk, ge], FP16, tag="b",
                                        bufs=ab_bufs)
                        nc.gpsimd.dma_gather(
                            out_ap=a_t[:], in_ap=xt[:, :ge],
                            idxs_ap=ia_all[:, csl],
                            num_idxs=ch_idx, num_idxs_reg=ch_idx, elem_size=ge,
                            elem_step=BATCH,
                            queue_num=(2 * chunk) % queues,
                            single_packet=single_packet)
                        nc.gpsimd.dma_gather(
                            out_ap=b_t[:], in_ap=xt[:, :ge],
                            idxs_ap=ib_all[:, csl],
                            num_idxs=ch_idx, num_idxs_reg=ch_idx, elem_size=ge,
                            elem_step=BATCH,
                            queue_num=(2 * chunk + 1) % queues,
                            single_packet=single_packet)
                    else:
                        a_t, b_t = a_fix, b_fix
                    if not compute:
                        if out_dma and gathers:
                            nc.sync.dma_start(
                                out=out[:, chunk * cols_per_chunk:(chunk + 1) * cols_per_chunk, :],
                                in_=a_t[:])
                        continue

                    t1 = pool.tile([P, cols_per_chunk, BATCH], FP16, tag="t1")
                    t2 = pool.tile([P, cols_per_chunk, BATCH], FP16, tag="t2")
                    for g in range(cols_per_chunk):
                        col = chunk * cols_per_chunk + g
                        a_sl = a_t[:, g, :]
                        nc.vector.tensor_scalar(
                            out=t1[:, g, :], in0=a_sl,
                            scalar1=k[3][:, col:col + 1],
                            scalar2=k[2][:, col:col + 1],
                            op0=mybir.AluOpType.mult, op1=mybir.AluOpType.add)
                        nc.vector.tensor_scalar(
                            out=t2[:, g, :], in0=a_sl,
                            scalar1=k[1][:, col:col + 1],
                            scalar2=k[0][:, col:col + 1],
                            op0=mybir.AluOpType.mult, op1=mybir.AluOpType.add)
                    nc.vector.tensor_tensor(out=t1[:], in0=t1[:], in1=b_t[:],
                                            op=mybir.AluOpType.mult)
                    m = t1

                    out_t = pool.tile([P, cols_per_chunk, BATCH], FP16, tag="out_t")
                    half_cols = max(1, cols_per_chunk // (4 if psum_fine else 2))
                    for h in range(cols_per_chunk // half_cols):
                        psum = ppool.tile([P, half_cols, BATCH], F32, tag="ps")
                        for g2 in range(half_cols):
                            g = h * half_cols + g2
                            nc.tensor.matmul(psum[:, g2, :], ident_sb[:],
                                             m[:, g, :], start=True, stop=False)
                            nc.tensor.matmul(psum[:, g2, :], ident_sb[:],
                                             t2[:, g, :], start=False, stop=True)
                        hs = slice(h * half_cols, (h + 1) * half_cols)
                        nc.scalar.activation(out_t[:, hs, :], psum[:],
                                             mybir.ActivationFunctionType.Copy)
                        if out_dma and out_split:
                            c0 = chunk * cols_per_chunk + h * half_cols
                            nc.sync.dma_start(
                                out=out[:, c0:c0 + half_cols, :],
                                in_=out_t[:, hs, :])
                    if out_dma and out_split:
                        pass  # emitted per half below
                    elif out_dma:
                        nc.sync.dma_start(
                            out=out[:, chunk * cols_per_chunk:(chunk + 1) * cols_per_chunk, :],
                            in_=out_t[:])
    nc.compile()
    return nc


def make_in_maps(x, weights, connections, ch_idx=CH_IDX):
    n_chunk = SHARD // ch_idx
    xt = np.ascontiguousarray(x.T.astype(np.float16))
    coef_dev = np.tile(OP_COEFFS.T.reshape(1, 64), (P, 1)).astype(np.float32)
    coef_dev = np.ascontiguousarray(coef_dev)
    ident = np.eye(P, dtype=np.float16)

    in_maps = []
    for c in range(N_CORES):
        base = c * SHARD
        w_shard = weights[base:base + SHARD]
        w_dev = np.ascontiguousarray(
            w_shard.reshape(NCOL, P, 16).transpose(1, 0, 2).reshape(P, NCOL * 16)
        ).astype(np.float16)

        conn = connections[base:base + SHARD].astype(np.int16)
        idx = []
        for j in range(2):
            arr = conn[:, j].reshape(n_chunk, ch_idx // 16, 16).transpose(0, 2, 1)
            arr = np.tile(arr, (1, 8, 1))
            idx.append(np.ascontiguousarray(
                arr.transpose(1, 0, 2).reshape(P, -1)))
        in_maps.append({
            "xt": xt, "w": w_dev, "coef": coef_dev, "ident": ident,
            "idxa": idx[0], "idxb": idx[1],
        })
    return in_maps


def assemble_output(results):
    shards = []
    for c in range(N_CORES):
        o = results[c]["out"]  # [P, NCOL, BATCH]; neuron col*128+p at [p, col]
        shards.append(o.transpose(1, 0, 2).reshape(SHARD, BATCH))
    full = np.concatenate(shards, axis=0)
    return np.ascontiguousarray(full.T.astype(np.float32))


FP8 = mybir.dt.float8e4


def build_program_fp8(n_reps: int = 1, queues: int = 4, t2_act_cols: int = 3,
                      ab_bufs: int = 3, work_bufs: int = 3,
                      gathers: bool = True, compute: bool = True,
                      out_dma: bool = True):
    """fp8-gather variant: x gathered as e4m3 (512B descs), PE identity-matmul
    casts fp8->psum f32; ACT computes t1=k3*a+k2 (and part of t2) during the
    psum->SBUF traversal; DVE does the rest of t2, m=t1*b, out=m+t2."""
    ch_idx = CH_IDX
    n_chunk = SHARD // ch_idx
    cols = ch_idx // P  # 8 cols per chunk
    nc = bacc.Bacc("TRN2", target_bir_lowering=False, debug=False,
                   num_devices=N_CORES, num_swdge_queues=queues)

    xt = nc.dram_tensor("xt", [IN_DIM, BATCH], FP8, kind="ExternalInput")
    w = nc.dram_tensor("w", [P, NCOL * 16], FP16, kind="ExternalInput")
    coef = nc.dram_tensor("coef", [P, 64], F32, kind="ExternalInput")
    ident = nc.dram_tensor("ident", [P, P], FP8, kind="ExternalInput")
    idxa = nc.dram_tensor("idxa", [P, SHARD // 16], I16, kind="ExternalInput")
    idxb = nc.dram_tensor("idxb", [P, SHARD // 16], I16, kind="ExternalInput")
    out = nc.dram_tensor("out", [P, NCOL, BATCH], FP16, kind="ExternalOutput")

    with tile.TileContext(nc) as tc:
        with tc.tile_pool(name="const", bufs=1) as cpool, \
             tc.tile_pool(name="work", bufs=work_bufs) as pool, \
             tc.tile_pool(name="psum", bufs=2, space="PSUM") as ppool:
            ident_sb = cpool.tile([P, P], FP8, tag="ident_sb")
            nc.sync.dma_start(out=ident_sb[:], in_=ident[:])
            coef_sb = cpool.tile([P, 64], F32, tag="coef_sb")
            nc.sync.dma_start(out=coef_sb[:], in_=coef[:])

            for _rep in range(n_reps):
                # ---- k coefficients: k = softmax(w) @ OP_COEFFS
                w_sb = cpool.tile([P, NCOL * 16], FP16, tag="w_sb", bufs=2)
                nc.sync.dma_start(out=w_sb[:], in_=w[:])
                e = cpool.tile([P, NCOL * 16], F32, tag="e", bufs=2)
                nc.scalar.activation(e[:], w_sb[:], mybir.ActivationFunctionType.Exp)
                e3 = e[:].rearrange("p (c i) -> p c i", i=16)

                s = cpool.tile([P, NCOL], F32, tag="s", bufs=2)
                nc.vector.tensor_reduce(out=s[:], in_=e3, axis=mybir.AxisListType.X,
                                        op=mybir.AluOpType.add)
                rs = cpool.tile([P, NCOL], F32, tag="rs", bufs=2)
                nc.vector.reciprocal(rs[:], s[:])

                k = []
                for cc in range(4):
                    m4 = cpool.tile([P, NCOL * 16], F32, tag=f"ktmp{cc}", bufs=2)
                    cb = coef_sb[:, cc * 16:(cc + 1) * 16].unsqueeze(1).broadcast_to(
                        [P, NCOL, 16])
                    nc.gpsimd.tensor_tensor(
                        out=m4[:].rearrange("p (c i) -> p c i", i=16),
                        in0=e3, in1=cb, op=mybir.AluOpType.mult)
                    ks = cpool.tile([P, NCOL], F32, tag=f"ksum{cc}", bufs=2)
                    nc.vector.tensor_reduce(
                        out=ks[:], in_=m4[:].rearrange("p (c i) -> p c i", i=16),
                        axis=mybir.AxisListType.X, op=mybir.AluOpType.add)
                    kc = cpool.tile([P, NCOL], F32, tag=f"k{cc}", bufs=2)
                    nc.vector.tensor_tensor(out=kc[:], in0=ks[:], in1=rs[:],
                                            op=mybir.AluOpType.mult)
                    k.append(kc)

                ia_all = cpool.tile([P, SHARD // 16], I16, tag="ia_all", bufs=2)
                ib_all = cpool.tile([P, SHARD // 16], I16, tag="ib_all", bufs=2)
                nc.sync.dma_start(out=ia_all[:], in_=idxa[:])
                nc.sync.dma_start(out=ib_all[:], in_=idxb[:])
                if not gathers and _rep == 0:
                    a8_fix = cpool.tile([P, cols, BATCH], FP8, tag="a8_fix")
                    b8_fix = cpool.tile([P, cols, BATCH], FP8, tag="b8_fix")
                    nc.vector.memset(a8_fix[:], 0.25)
                    nc.vector.memset(b8_fix[:], 0.25)

                for chunk in range(n_chunk):
                    csl = slice(chunk * (ch_idx // 16), (chunk + 1) * (ch_idx // 16))
                    if gathers:
                        a8 = pool.tile([P, cols, BATCH], FP8, tag="a8",
                                       bufs=ab_bufs)
                        b8 = pool.tile([P, cols, BATCH], FP8, tag="b8",
                                       bufs=ab_bufs)
                    else:
                        a8, b8 = a8_fix, b8_fix
                    if gathers:
                        nc.gpsimd.dma_gather(
                            out_ap=a8[:], in_ap=xt[:], idxs_ap=ia_all[:, csl],
                            num_idxs=ch_idx, num_idxs_reg=ch_idx, elem_size=BATCH,
                            queue_num=(2 * chunk) % queues)
                        nc.gpsimd.dma_gather(
                            out_ap=b8[:], in_ap=xt[:], idxs_ap=ib_all[:, csl],
                            num_idxs=ch_idx, num_idxs_reg=ch_idx, elem_size=BATCH,
                            queue_num=(2 * chunk + 1) % queues)
                    if not compute:
                        if out_dma and gathers:
                            nc.sync.dma_start(
                                out=out[:, chunk * cols:(chunk + 1) * cols, :256],
                                in_=a8[:].bitcast(FP16))
                        continue

                    t1 = pool.tile([P, cols, BATCH], FP16, tag="t1")
                    t2 = pool.tile([P, cols, BATCH], FP16, tag="t2")
                    b16 = pool.tile([P, cols, BATCH], FP16, tag="b16")
                    for q in range(cols // 2):
                        psa = ppool.tile([P, 2, BATCH], F32, tag="psa")
                        psb = ppool.tile([P, 2, BATCH], F32, tag="psb")
                        for j in range(2):
                            lc = 2 * q + j
                            nc.tensor.matmul(psa[:, j, :], ident_sb[:],
                                             a8[:, lc, :], start=True, stop=True)
                            nc.tensor.matmul(psb[:, j, :], ident_sb[:],
                                             b8[:, lc, :], start=True, stop=True)
                        nc.scalar.activation(b16[:, 2 * q:2 * q + 2, :], psb[:],
                                             mybir.ActivationFunctionType.Copy)
                        for j in range(2):
                            lc = 2 * q + j
                            col = chunk * cols + lc
                            nc.scalar.activation(
                                t1[:, lc, :], psa[:, j, :],
                                mybir.ActivationFunctionType.Identity,
                                scale=k[3][:, col:col + 1],
                                bias=k[2][:, col:col + 1])
                            if lc < t2_act_cols:
                                nc.scalar.activation(
                                    t2[:, lc, :], psa[:, j, :],
                                    mybir.ActivationFunctionType.Identity,
                                    scale=k[1][:, col:col + 1],
                                    bias=k[0][:, col:col + 1])
                            else:
                                nc.vector.tensor_scalar(
                                    out=t2[:, lc, :], in0=psa[:, j, :],
                                    scalar1=k[1][:, col:col + 1],
                                    scalar2=k[0][:, col:col + 1],
                                    op0=mybir.AluOpType.mult,
                                    op1=mybir.AluOpType.add)
                    nc.vector.tensor_tensor(out=t1[:], in0=t1[:], in1=b16[:],
                                            op=mybir.AluOpType.mult)
                    out_t = pool.tile([P, cols, BATCH], FP16, tag="out_t")
                    nc.vector.tensor_tensor(out=out_t[:], in0=t1[:], in1=t2[:],
                                            op=mybir.AluOpType.add)
                    if out_dma:
                        nc.sync.dma_start(
                            out=out[:, chunk * cols:(chunk + 1) * cols, :],
                            in_=out_t[:])
    nc.compile()
    return nc


def make_in_maps_fp8(x, weights, connections, ch_idx=CH_IDX):
    n_chunk = SHARD // ch_idx
    xt8 = np.ascontiguousarray(x.T).astype(mybir.dt.np(FP8))
    coef_dev = np.tile(OP_COEFFS.T.reshape(1, 64), (P, 1)).astype(np.float32)
    coef_dev = np.ascontiguousarray(coef_dev)
    ident = np.eye(P).astype(mybir.dt.np(FP8))

    in_maps = []
    for c in range(N_CORES):
        base = c * SHARD
        w_shard = weights[base:base + SHARD]
        w_dev = np.ascontiguousarray(
            w_shard.reshape(NCOL, P, 16).transpose(1, 0, 2).reshape(P, NCOL * 16)
        ).astype(np.float16)

        conn = connections[base:base + SHARD].astype(np.int16)
        idx = []
        for j in range(2):
            arr = conn[:, j].reshape(n_chunk, ch_idx // 16, 16).transpose(0, 2, 1)
            arr = np.tile(arr, (1, 8, 1))
            idx.append(np.ascontiguousarray(
                arr.transpose(1, 0, 2).reshape(P, -1)))
        in_maps.append({
            "xt": xt8, "w": w_dev, "coef": coef_dev, "ident": ident,
            "idxa": idx[0], "idxb": idx[1],
        })
    return in_maps


def build_program_e(n_reps: int = 1, queues: int = 4,
                    t1_eng: str = 'act', t2_eng: str = 'act',
                    bcast_eng: str = 'dve',
                    ab_bufs: int = 3, work_bufs: int = 3,
                    gathers: bool = True, compute: bool = True,
                    out_dma: bool = True):
    """Mixed-precision gather variant: a gathered fp16 (1KB descs), b gathered
    fp8 e4m3 (512B descs).  No PE/PSUM: b8 cast to fp16 by a DVE/ACT copy;
    t1 = k3*a+k2 and t2 = k1*a+k0 from fp16 a; m = t1*b16; out = m + t2."""
    ch_idx = CH_IDX
    n_chunk = SHARD // ch_idx
    cols = ch_idx // P
    nc = bacc.Bacc("TRN2", target_bir_lowering=False, debug=False,
                   num_devices=N_CORES, num_swdge_queues=queues)

    xta = nc.dram_tensor("xta", [IN_DIM, BATCH], FP16, kind="ExternalInput")
    xtb = nc.dram_tensor("xtb", [IN_DIM, BATCH], FP8, kind="ExternalInput")
    w = nc.dram_tensor("w", [P, NCOL * 16], FP16, kind="ExternalInput")
    coef = nc.dram_tensor("coef", [P, 64], F32, kind="ExternalInput")
    idxa = nc.dram_tensor("idxa", [P, SHARD // 16], I16, kind="ExternalInput")
    idxb = nc.dram_tensor("idxb", [P, SHARD // 16], I16, kind="ExternalInput")
    out = nc.dram_tensor("out", [P, NCOL, BATCH], FP16, kind="ExternalOutput")

    with tile.TileContext(nc) as tc:
        with tc.tile_pool(name="const", bufs=1) as cpool, \
             tc.tile_pool(name="work", bufs=work_bufs) as pool:
            coef_sb = cpool.tile([P, 64], F32, tag="coef_sb")
            nc.sync.dma_start(out=coef_sb[:], in_=coef[:])

            for _rep in range(n_reps):
                # ---- k coefficients: k = softmax(w) @ OP_COEFFS
                w_sb = cpool.tile([P, NCOL * 16], FP16, tag="w_sb", bufs=2)
                nc.sync.dma_start(out=w_sb[:], in_=w[:])
                e = cpool.tile([P, NCOL * 16], F32, tag="e", bufs=2)
                nc.scalar.activation(e[:], w_sb[:], mybir.ActivationFunctionType.Exp)
                e3 = e[:].rearrange("p (c i) -> p c i", i=16)

                s = cpool.tile([P, NCOL], F32, tag="s", bufs=2)
                nc.vector.tensor_reduce(out=s[:], in_=e3, axis=mybir.AxisListType.X,
                                        op=mybir.AluOpType.add)
                rs = cpool.tile([P, NCOL], F32, tag="rs", bufs=2)
                nc.vector.reciprocal(rs[:], s[:])

                k = []
                for cc in range(4):
                    m4 = cpool.tile([P, NCOL * 16], F32, tag=f"ktmp{cc}", bufs=2)
                    cb = coef_sb[:, cc * 16:(cc + 1) * 16].unsqueeze(1).broadcast_to(
                        [P, NCOL, 16])
                    nc.gpsimd.tensor_tensor(
                        out=m4[:].rearrange("p (c i) -> p c i", i=16),
                        in0=e3, in1=cb, op=mybir.AluOpType.mult)
                    ks = cpool.tile([P, NCOL], F32, tag=f"ksum{cc}", bufs=2)
                    nc.vector.tensor_reduce(
                        out=ks[:], in_=m4[:].rearrange("p (c i) -> p c i", i=16),
                        axis=mybir.AxisListType.X, op=mybir.AluOpType.add)
                    kc = cpool.tile([P, NCOL], F32, tag=f"k{cc}", bufs=2)
                    nc.vector.tensor_tensor(out=kc[:], in0=ks[:], in1=rs[:],
                                            op=mybir.AluOpType.mult)
                    k.append(kc)

                ia_all = cpool.tile([P, SHARD // 16], I16, tag="ia_all", bufs=2)
                ib_all = cpool.tile([P, SHARD // 16], I16, tag="ib_all", bufs=2)
                nc.sync.dma_start(out=ia_all[:], in_=idxa[:])
                nc.sync.dma_start(out=ib_all[:], in_=idxb[:])
                if not gathers and _rep == 0:
                    a_fix = cpool.tile([P, cols, BATCH], FP16, tag="a_fix")
                    b8_fix = cpool.tile([P, cols, BATCH], FP8, tag="b8_fix")
                    nc.vector.memset(a_fix[:], 0.25)
                    nc.vector.memset(b8_fix[:], 0.25)

                for chunk in range(n_chunk):
                    csl = slice(chunk * (ch_idx // 16), (chunk + 1) * (ch_idx // 16))
                    if gathers:
                        a16 = pool.tile([P, cols, BATCH], FP16, tag="a16",
                                        bufs=ab_bufs)
                        b8 = pool.tile([P, cols, BATCH], FP8, tag="b8",
                                       bufs=ab_bufs)
                        nc.gpsimd.dma_gather(
                            out_ap=a16[:], in_ap=xta[:], idxs_ap=ia_all[:, csl],
                            num_idxs=ch_idx, num_idxs_reg=ch_idx, elem_size=BATCH,
                            queue_num=(2 * chunk) % queues)
                        nc.gpsimd.dma_gather(
                            out_ap=b8[:], in_ap=xtb[:], idxs_ap=ib_all[:, csl],
                            num_idxs=ch_idx, num_idxs_reg=ch_idx, elem_size=BATCH,
                            queue_num=(2 * chunk + 1) % queues)
                    else:
                        a16, b8 = a_fix, b8_fix
                    if not compute:
                        if out_dma and gathers:
                            nc.sync.dma_start(
                                out=out[:, chunk * cols:(chunk + 1) * cols, :],
                                in_=a16[:])
                        continue

                    b16 = pool.tile([P, cols, BATCH], FP16, tag="b16")
                    if bcast_eng == 'dve':
                        nc.vector.tensor_copy(out=b16[:], in_=b8[:])
                    else:
                        nc.scalar.activation(b16[:], b8[:],
                                             mybir.ActivationFunctionType.Copy)
                    t1 = pool.tile([P, cols, BATCH], FP16, tag="t1")
                    t2 = pool.tile([P, cols, BATCH], FP16, tag="t2")
                    for g in range(cols):
                        col = chunk * cols + g
                        for dst, khi, klo, eng in ((t1, 3, 2, t1_eng),
                                                   (t2, 1, 0, t2_eng)):
                            if eng == 'act':
                                nc.scalar.activation(
                                    dst[:, g, :], a16[:, g, :],
                                    mybir.ActivationFunctionType.Identity,
                                    scale=k[khi][:, col:col + 1],
                                    bias=k[klo][:, col:col + 1])
                            else:
                                nc.vector.tensor_scalar(
                                    out=dst[:, g, :], in0=a16[:, g, :],
                                    scalar1=k[khi][:, col:col + 1],
                                    scalar2=k[klo][:, col:col + 1],
                                    op0=mybir.AluOpType.mult,
                                    op1=mybir.AluOpType.add)
                    nc.vector.tensor_tensor(out=t1[:], in0=t1[:], in1=b16[:],
                                            op=mybir.AluOpType.mult)
                    out_t = pool.tile([P, cols, BATCH], FP16, tag="out_t")
                    nc.vector.tensor_tensor(out=out_t[:], in0=t1[:], in1=t2[:],
                                            op=mybir.AluOpType.add)
                    if out_dma:
                        nc.sync.dma_start(
                            out=out[:, chunk * cols:(chunk + 1) * cols, :],
                            in_=out_t[:])
    nc.compile()
    return nc


def make_in_maps_e(x, weights, connections, ch_idx=CH_IDX):
    n_chunk = SHARD // ch_idx
    xt = np.ascontiguousarray(x.T)
    xta = xt.astype(np.float16)
    xtb = xt.astype(mybir.dt.np(FP8))
    coef_dev = np.tile(OP_COEFFS.T.reshape(1, 64), (P, 1)).astype(np.float32)
    coef_dev = np.ascontiguousarray(coef_dev)

    in_maps = []
    for c in range(N_CORES):
        base = c * SHARD
        w_shard = weights[base:base + SHARD]
        w_dev = np.ascontiguousarray(
            w_shard.reshape(NCOL, P, 16).transpose(1, 0, 2).reshape(P, NCOL * 16)
        ).astype(np.float16)

        conn = connections[base:base + SHARD].astype(np.int16)
        idx = []
        for j in range(2):
            arr = conn[:, j].reshape(n_chunk, ch_idx // 16, 16).transpose(0, 2, 1)
            arr = np.tile(arr, (1, 8, 1))
            idx.append(np.ascontiguousarray(
                arr.transpose(1, 0, 2).reshape(P, -1)))
        in_maps.append({
            "xta": xta, "xtb": xtb, "w": w_dev, "coef": coef_dev,
            "idxa": idx[0], "idxb": idx[1],
        })
    return in_maps


def build_program_f2(n_reps: int = 1, queues: int = 4, t2_act_cols: int = 4,
                     ab_bufs: int = 3, work_bufs: int = 3,
                     kmult_engine: str = 'dve',
                     gathers: bool = True, compute: bool = True,
                     out_dma: bool = True):
    """Both-fp8 gathers; PE identity-casts a8/b8 into PSUM quarters; ACT
    evacuates psum->fp16 (plain Copy); DVE does t1/t2 (tensor_scalar 4x on
    fp16) with t2_act_cols of t2 moved to ACT Identity, then m=t1*b16 and
    out=m+t2; one HWDGE store per chunk."""
    ch_idx = CH_IDX
    n_chunk = SHARD // ch_idx
    cols = ch_idx // P
    nc = bacc.Bacc("TRN2", target_bir_lowering=False, debug=False,
                   num_devices=N_CORES, num_swdge_queues=queues)

    xt = nc.dram_tensor("xt", [IN_DIM, BATCH], FP8, kind="ExternalInput")
    w = nc.dram_tensor("w", [P, NCOL * 16], FP16, kind="ExternalInput")
    coef = nc.dram_tensor("coef", [P, 64], F32, kind="ExternalInput")
    ident = nc.dram_tensor("ident", [P, P], FP8, kind="ExternalInput")
    idxa = nc.dram_tensor("idxa", [P, SHARD // 16], I16, kind="ExternalInput")
    idxb = nc.dram_tensor("idxb", [P, SHARD // 16], I16, kind="ExternalInput")
    out = nc.dram_tensor("out", [P, NCOL, BATCH], FP16, kind="ExternalOutput")

    with tile.TileContext(nc) as tc:
        with tc.tile_pool(name="const", bufs=1) as cpool, \
             tc.tile_pool(name="work", bufs=work_bufs) as pool, \
             tc.tile_pool(name="psum", bufs=2, space="PSUM") as ppool:
            ident_sb = cpool.tile([P, P], FP8, tag="ident_sb")
            nc.sync.dma_start(out=ident_sb[:], in_=ident[:])
            coef_sb = cpool.tile([P, 64], F32, tag="coef_sb")
            nc.sync.dma_start(out=coef_sb[:], in_=coef[:])

            for _rep in range(n_reps):
                # ---- k coefficients: k = softmax(w) @ OP_COEFFS
                w_sb = cpool.tile([P, NCOL * 16], FP16, tag="w_sb", bufs=2)
                nc.sync.dma_start(out=w_sb[:], in_=w[:])
                e = cpool.tile([P, NCOL * 16], F32, tag="e", bufs=2)
                nc.scalar.activation(e[:], w_sb[:], mybir.ActivationFunctionType.Exp)
                e3 = e[:].rearrange("p (c i) -> p c i", i=16)

                s = cpool.tile([P, NCOL], F32, tag="s", bufs=2)
                nc.vector.tensor_reduce(out=s[:], in_=e3, axis=mybir.AxisListType.X,
                                        op=mybir.AluOpType.add)
                rs = cpool.tile([P, NCOL], F32, tag="rs", bufs=2)
                nc.vector.reciprocal(rs[:], s[:])

                k = []
                for cc in range(4):
                    m4 = cpool.tile([P, NCOL * 16], F32, tag=f"ktmp{cc}", bufs=2)
                    cb = coef_sb[:, cc * 16:(cc + 1) * 16].unsqueeze(1).broadcast_to(
                        [P, NCOL, 16])
                    keng = nc.gpsimd if kmult_engine == 'pool' else nc.vector
                    keng.tensor_tensor(
                        out=m4[:].rearrange("p (c i) -> p c i", i=16),
                        in0=e3, in1=cb, op=mybir.AluOpType.mult)
                    ks = cpool.tile([P, NCOL], F32, tag=f"ksum{cc}", bufs=2)
                    nc.vector.tensor_reduce(
                        out=ks[:], in_=m4[:].rearrange("p (c i) -> p c i", i=16),
                        axis=mybir.AxisListType.X, op=mybir.AluOpType.add)
                    kc = cpool.tile([P, NCOL], F32, tag=f"k{cc}", bufs=2)
                    nc.vector.tensor_tensor(out=kc[:], in0=ks[:], in1=rs[:],
                                            op=mybir.AluOpType.mult)
                    k.append(kc)

                ia_all = cpool.tile([P, SHARD // 16], I16, tag="ia_all", bufs=2)
                ib_all = cpool.tile([P, SHARD // 16], I16, tag="ib_all", bufs=2)
                nc.sync.dma_start(out=ia_all[:], in_=idxa[:])
                nc.sync.dma_start(out=ib_all[:], in_=idxb[:])
                if not gathers and _rep == 0:
                    a8_fix = cpool.tile([P, cols, BATCH], FP8, tag="a8_fix")
                    b8_fix = cpool.tile([P, cols, BATCH], FP8, tag="b8_fix")
                    nc.vector.memset(a8_fix[:], 0.25)
                    nc.vector.memset(b8_fix[:], 0.25)

                for chunk in range(n_chunk):
                    csl = slice(chunk * (ch_idx // 16), (chunk + 1) * (ch_idx // 16))
                    if gathers:
                        a8 = pool.tile([P, cols, BATCH], FP8, tag="a8",
                                       bufs=ab_bufs)
                        b8 = pool.tile([P, cols, BATCH], FP8, tag="b8",
                                       bufs=ab_bufs)
                        nc.gpsimd.dma_gather(
                            out_ap=a8[:], in_ap=xt[:], idxs_ap=ia_all[:, csl],
                            num_idxs=ch_idx, num_idxs_reg=ch_idx, elem_size=BATCH,
                            queue_num=(2 * chunk) % queues)
                        nc.gpsimd.dma_gather(
                            out_ap=b8[:], in_ap=xt[:], idxs_ap=ib_all[:, csl],
                            num_idxs=ch_idx, num_idxs_reg=ch_idx, elem_size=BATCH,
                            queue_num=(2 * chunk + 1) % queues)
                    else:
                        a8, b8 = a8_fix, b8_fix
                    if not compute:
                        if out_dma and gathers:
                            nc.sync.dma_start(
                                out=out[:, chunk * cols:(chunk + 1) * cols, :256],
                                in_=a8[:].bitcast(FP16))
                        continue

                    b16 = pool.tile([P, cols, BATCH], FP16, tag="b16")
                    if not a_direct:
                        a16 = pool.tile([P, cols, BATCH], FP16, tag="a16")
                    for q in range(cols // 2):
                        psb = ppool.tile([P, 2, BATCH], F32, tag="psb")
                        for j in range(2):
                            lc = 2 * q + j
                            nc.tensor.matmul(psb[:, j, :], ident_sb[:],
                                             b8[:, lc, :], start=True, stop=True)
                        nc.scalar.activation(b16[:, 2 * q:2 * q + 2, :], psb[:],
                                             mybir.ActivationFunctionType.Copy)
                        if not a_direct:
                            psa = ppool.tile([P, 2, BATCH], F32, tag="psa")
                            for j in range(2):
                                lc = 2 * q + j
                                nc.tensor.matmul(psa[:, j, :], ident_sb[:],
                                                 a8[:, lc, :], start=True,
                                                 stop=True)
                            nc.scalar.activation(
                                a16[:, 2 * q:2 * q + 2, :], psa[:],
                                mybir.ActivationFunctionType.Copy)
                    a_src = a8 if a_direct else a16
                    t1 = pool.tile([P, cols, BATCH], FP16, tag="t1")
                    t2 = pool.tile([P, cols, BATCH], FP16, tag="t2")
                    for g in range(cols):
                        col = chunk * cols + g
                        nc.vector.tensor_scalar(
                            out=t1[:, g, :], in0=a_src[:, g, :],
                            scalar1=k[3][:, col:col + 1],
                            scalar2=k[2][:, col:col + 1],
                            op0=mybir.AluOpType.mult, op1=mybir.AluOpType.add)
                        if g < t2_act_cols:
                            nc.scalar.activation(
                                t2[:, g, :], a_src[:, g, :],
                                mybir.ActivationFunctionType.Identity,
                                scale=k[1][:, col:col + 1],
                                bias=k[0][:, col:col + 1])
                        else:
                            nc.vector.tensor_scalar(
                                out=t2[:, g, :], in0=a_src[:, g, :],
                                scalar1=k[1][:, col:col + 1],
                                scalar2=k[0][:, col:col + 1],
                                op0=mybir.AluOpType.mult,
                                op1=mybir.AluOpType.add)
                    nc.vector.tensor_tensor(out=t1[:], in0=t1[:], in1=b16[:],
                                            op=mybir.AluOpType.mult)
                    out_t = pool.tile([P, cols, BATCH], FP16, tag="out_t")
                    nc.vector.tensor_tensor(out=out_t[:], in0=t1[:], in1=t2[:],
                                            op=mybir.AluOpType.add)
                    if out_dma:
                        nc.sync.dma_start(
                            out=out[:, chunk * cols:(chunk + 1) * cols, :],
                            in_=out_t[:])
    nc.compile()
    return nc


def build_program_f2k(n_reps: int = 1, queues: int = 4, t2_act_cols: int = 8,
                      ab_bufs: int = 3, work_bufs: int = 3,
                      kmult_engine: str = 'dve', store_halves: bool = False,
                      a_direct: bool = False,
                      gathers: bool = True, compute: bool = True,
                      out_dma: bool = True):
    """f2 (t2-on-ACT) with the k-coefficient phase software-pipelined one rep
    ahead: rep N's chunk loop consumes k computed during rep N-1, so the
    exp/reduce/mult chain never sits on the critical path between gather
    batches."""
    ch_idx = CH_IDX
    n_chunk = SHARD // ch_idx
    cols = ch_idx // P
    nc = bacc.Bacc("TRN2", target_bir_lowering=False, debug=False,
                   num_devices=N_CORES, num_swdge_queues=queues)

    xt = nc.dram_tensor("xt", [IN_DIM, BATCH], FP8, kind="ExternalInput")
    w = nc.dram_tensor("w", [P, NCOL * 16], FP16, kind="ExternalInput")
    coef = nc.dram_tensor("coef", [P, 64], F32, kind="ExternalInput")
    ident = nc.dram_tensor("ident", [P, P], FP8, kind="ExternalInput")
    idxa = nc.dram_tensor("idxa", [P, SHARD // 16], I16, kind="ExternalInput")
    idxb = nc.dram_tensor("idxb", [P, SHARD // 16], I16, kind="ExternalInput")
    out = nc.dram_tensor("out", [P, NCOL, BATCH], FP16, kind="ExternalOutput")

    with tile.TileContext(nc) as tc:
        with tc.tile_pool(name="const", bufs=1) as cpool, \
             tc.tile_pool(name="work", bufs=work_bufs) as pool, \
             tc.tile_pool(name="psum", bufs=2, space="PSUM") as ppool:
            ident_sb = cpool.tile([P, P], FP8, tag="ident_sb")
            nc.sync.dma_start(out=ident_sb[:], in_=ident[:])
            coef_sb = cpool.tile([P, 64], F32, tag="coef_sb")
            nc.sync.dma_start(out=coef_sb[:], in_=coef[:])

            def k_phase():
                w_sb = cpool.tile([P, NCOL * 16], FP16, tag="w_sb", bufs=2)
                nc.sync.dma_start(out=w_sb[:], in_=w[:])
                e = cpool.tile([P, NCOL * 16], F32, tag="e", bufs=2)
                nc.scalar.activation(e[:], w_sb[:],
                                     mybir.ActivationFunctionType.Exp)
                e3 = e[:].rearrange("p (c i) -> p c i", i=16)
                s = cpool.tile([P, NCOL], F32, tag="s", bufs=2)
                nc.vector.tensor_reduce(out=s[:], in_=e3,
                                        axis=mybir.AxisListType.X,
                                        op=mybir.AluOpType.add)
                rs = cpool.tile([P, NCOL], F32, tag="rs", bufs=2)
                nc.vector.reciprocal(rs[:], s[:])
                k = []
                for cc in range(4):
                    m4 = cpool.tile([P, NCOL * 16], F32, tag=f"ktmp{cc}", bufs=2)
                    cb = coef_sb[:, cc * 16:(cc + 1) * 16].unsqueeze(1)
                    cb = cb.broadcast_to([P, NCOL, 16])
                    keng = nc.gpsimd if kmult_engine == 'pool' else nc.vector
                    keng.tensor_tensor(
                        out=m4[:].rearrange("p (c i) -> p c i", i=16),
                        in0=e3, in1=cb, op=mybir.AluOpType.mult)
                    ks = cpool.tile([P, NCOL], F32, tag=f"ksum{cc}", bufs=2)
                    nc.vector.tensor_reduce(
                        out=ks[:], in_=m4[:].rearrange("p (c i) -> p c i", i=16),
                        axis=mybir.AxisListType.X, op=mybir.AluOpType.add)
                    kc = cpool.tile([P, NCOL], F32, tag=f"k{cc}", bufs=2)
                    nc.vector.tensor_tensor(out=kc[:], in0=ks[:], in1=rs[:],
                                            op=mybir.AluOpType.mult)
                    k.append(kc)
                return k

            k = k_phase()  # prologue: k for the first rep
            for _rep in range(n_reps):
                ia_all = cpool.tile([P, SHARD // 16], I16, tag="ia_all", bufs=2)
                ib_all = cpool.tile([P, SHARD // 16], I16, tag="ib_all", bufs=2)
                nc.sync.dma_start(out=ia_all[:], in_=idxa[:])
                nc.sync.dma_start(out=ib_all[:], in_=idxb[:])
                if not gathers and _rep == 0:
                    a8_fix = cpool.tile([P, cols, BATCH], FP8, tag="a8_fix")
                    b8_fix = cpool.tile([P, cols, BATCH], FP8, tag="b8_fix")
                    nc.vector.memset(a8_fix[:], 0.25)
                    nc.vector.memset(b8_fix[:], 0.25)

                for chunk in range(n_chunk):
                    csl = slice(chunk * (ch_idx // 16), (chunk + 1) * (ch_idx // 16))
                    if gathers:
                        a8 = pool.tile([P, cols, BATCH], FP8, tag="a8",
                                       bufs=ab_bufs)
                        b8 = pool.tile([P, cols, BATCH], FP8, tag="b8",
                                       bufs=ab_bufs)
                        nc.gpsimd.dma_gather(
                            out_ap=a8[:], in_ap=xt[:], idxs_ap=ia_all[:, csl],
                            num_idxs=ch_idx, num_idxs_reg=ch_idx, elem_size=BATCH,
                            queue_num=(2 * chunk) % queues)
                        nc.gpsimd.dma_gather(
                            out_ap=b8[:], in_ap=xt[:], idxs_ap=ib_all[:, csl],
                            num_idxs=ch_idx, num_idxs_reg=ch_idx, elem_size=BATCH,
                            queue_num=(2 * chunk + 1) % queues)
                    else:
                        a8, b8 = a8_fix, b8_fix
                    if not compute:
                        if out_dma and gathers:
                            nc.sync.dma_start(
                                out=out[:, chunk * cols:(chunk + 1) * cols, :256],
                                in_=a8[:].bitcast(FP16))
                        continue

                    b16 = pool.tile([P, cols, BATCH], FP16, tag="b16")
                    if not a_direct:
                        a16 = pool.tile([P, cols, BATCH], FP16, tag="a16")
                    for q in range(cols // 2):
                        psb = ppool.tile([P, 2, BATCH], F32, tag="psb")
                        for j in range(2):
                            lc = 2 * q + j
                            nc.tensor.matmul(psb[:, j, :], ident_sb[:],
                                             b8[:, lc, :], start=True, stop=True)
                        nc.scalar.activation(b16[:, 2 * q:2 * q + 2, :], psb[:],
                                             mybir.ActivationFunctionType.Copy)
                        if not a_direct:
                            psa = ppool.tile([P, 2, BATCH], F32, tag="psa")
                            for j in range(2):
                                lc = 2 * q + j
                                nc.tensor.matmul(psa[:, j, :], ident_sb[:],
                                                 a8[:, lc, :], start=True,
                                                 stop=True)
                            nc.scalar.activation(
                                a16[:, 2 * q:2 * q + 2, :], psa[:],
                                mybir.ActivationFunctionType.Copy)
                    a_src = a8 if a_direct else a16
                    t1 = pool.tile([P, cols, BATCH], FP16, tag="t1")
                    t2 = pool.tile([P, cols, BATCH], FP16, tag="t2")
                    for g in range(cols):
                        col = chunk * cols + g
                        nc.vector.tensor_scalar(
                            out=t1[:, g, :], in0=a_src[:, g, :],
                            scalar1=k[3][:, col:col + 1],
                            scalar2=k[2][:, col:col + 1],
                            op0=mybir.AluOpType.mult, op1=mybir.AluOpType.add)
                        if g < t2_act_cols:
                            nc.scalar.activation(
                                t2[:, g, :], a_src[:, g, :],
                                mybir.ActivationFunctionType.Identity,
                                scale=k[1][:, col:col + 1],
                                bias=k[0][:, col:col + 1])
                        else:
                            nc.vector.tensor_scalar(
                                out=t2[:, g, :], in0=a_src[:, g, :],
                                scalar1=k[1][:, col:col + 1],
                                scalar2=k[0][:, col:col + 1],
                                op0=mybir.AluOpType.mult,
                                op1=mybir.AluOpType.add)
                    out_t = pool.tile([P, cols, BATCH], FP16, tag="out_t")
                    if store_halves:
                        for h in range(2):
                            hs = slice(4 * h, 4 * h + 4)
                            nc.vector.tensor_tensor(
                                out=t1[:, hs, :], in0=t1[:, hs, :],
                                in1=b16[:, hs, :], op=mybir.AluOpType.mult)
                            nc.vector.tensor_tensor(
                                out=out_t[:, hs, :], in0=t1[:, hs, :],
                                in1=t2[:, hs, :], op=mybir.AluOpType.add)
                            if out_dma:
                                c0 = chunk * cols + 4 * h
                                nc.sync.dma_start(out=out[:, c0:c0 + 4, :],
                                                  in_=out_t[:, hs, :])
                    else:
                        nc.vector.tensor_tensor(out=t1[:], in0=t1[:],
                                                in1=b16[:],
                                                op=mybir.AluOpType.mult)
                        nc.vector.tensor_tensor(out=out_t[:], in0=t1[:],
                                                in1=t2[:],
                                                op=mybir.AluOpType.add)
                        if out_dma:
                            nc.sync.dma_start(
                                out=out[:, chunk * cols:(chunk + 1) * cols, :],
                                in_=out_t[:])
                if compute and _rep < n_reps - 1:
                    k = k_phase()  # k for the next rep, off the critical path
    nc.compile()
    return nc


def build_program_f2b(n_reps: int = 1, queues: int = 4, t2_act_cols: int = 8,
                      ab_bufs: int = 3, work_bufs: int = 3,
                      kmult_engine: str = 'dve', store_halves: bool = True,
                      gathers: bool = True, compute: bool = True,
                      out_dma: bool = True):
    """f2 with quarter-interleaved emission: per 2-col quarter do
    PE casts -> ACT evac -> t1/t2 -> DVE m -> DVE add, store per half-chunk.
    Shorter dependency chains let chunks pipeline tighter."""
    ch_idx = CH_IDX
    n_chunk = SHARD // ch_idx
    cols = ch_idx // P
    nc = bacc.Bacc("TRN2", target_bir_lowering=False, debug=False,
                   num_devices=N_CORES, num_swdge_queues=queues)

    xt = nc.dram_tensor("xt", [IN_DIM, BATCH], FP8, kind="ExternalInput")
    w = nc.dram_tensor("w", [P, NCOL * 16], FP16, kind="ExternalInput")
    coef = nc.dram_tensor("coef", [P, 64], F32, kind="ExternalInput")
    ident = nc.dram_tensor("ident", [P, P], FP8, kind="ExternalInput")
    idxa = nc.dram_tensor("idxa", [P, SHARD // 16], I16, kind="ExternalInput")
    idxb = nc.dram_tensor("idxb", [P, SHARD // 16], I16, kind="ExternalInput")
    out = nc.dram_tensor("out", [P, NCOL, BATCH], FP16, kind="ExternalOutput")

    with tile.TileContext(nc) as tc:
        with tc.tile_pool(name="const", bufs=1) as cpool, \
             tc.tile_pool(name="work", bufs=work_bufs) as pool, \
             tc.tile_pool(name="psum", bufs=2, space="PSUM") as ppool:
            ident_sb = cpool.tile([P, P], FP8, tag="ident_sb")
            nc.sync.dma_start(out=ident_sb[:], in_=ident[:])
            coef_sb = cpool.tile([P, 64], F32, tag="coef_sb")
            nc.sync.dma_start(out=coef_sb[:], in_=coef[:])

            for _rep in range(n_reps):
                w_sb = cpool.tile([P, NCOL * 16], FP16, tag="w_sb", bufs=2)
                nc.sync.dma_start(out=w_sb[:], in_=w[:])
                e = cpool.tile([P, NCOL * 16], F32, tag="e", bufs=2)
                nc.scalar.activation(e[:], w_sb[:], mybir.ActivationFunctionType.Exp)
                e3 = e[:].rearrange("p (c i) -> p c i", i=16)

                s = cpool.tile([P, NCOL], F32, tag="s", bufs=2)
                nc.vector.tensor_reduce(out=s[:], in_=e3, axis=mybir.AxisListType.X,
                                        op=mybir.AluOpType.add)
                rs = cpool.tile([P, NCOL], F32, tag="rs", bufs=2)
                nc.vector.reciprocal(rs[:], s[:])

                k = []
                for cc in range(4):
                    m4 = cpool.tile([P, NCOL * 16], F32, tag=f"ktmp{cc}", bufs=2)
                    cb = coef_sb[:, cc * 16:(cc + 1) * 16].unsqueeze(1).broadcast_to(
                        [P, NCOL, 16])
                    keng = nc.gpsimd if kmult_engine == 'pool' else nc.vector
                    keng.tensor_tensor(
                        out=m4[:].rearrange("p (c i) -> p c i", i=16),
                        in0=e3, in1=cb, op=mybir.AluOpType.mult)
                    ks = cpool.tile([P, NCOL], F32, tag=f"ksum{cc}", bufs=2)
                    nc.vector.tensor_reduce(
                        out=ks[:], in_=m4[:].rearrange("p (c i) -> p c i", i=16),
                        axis=mybir.AxisListType.X, op=mybir.AluOpType.add)
                    kc = cpool.tile([P, NCOL], F32, tag=f"k{cc}", bufs=2)
                    nc.vector.tensor_tensor(out=kc[:], in0=ks[:], in1=rs[:],
                                            op=mybir.AluOpType.mult)
                    k.append(kc)

                ia_all = cpool.tile([P, SHARD // 16], I16, tag="ia_all", bufs=2)
                ib_all = cpool.tile([P, SHARD // 16], I16, tag="ib_all", bufs=2)
                nc.sync.dma_start(out=ia_all[:], in_=idxa[:])
                nc.sync.dma_start(out=ib_all[:], in_=idxb[:])
                if not gathers and _rep == 0:
                    a8_fix = cpool.tile([P, cols, BATCH], FP8, tag="a8_fix")
                    b8_fix = cpool.tile([P, cols, BATCH], FP8, tag="b8_fix")
                    nc.vector.memset(a8_fix[:], 0.25)
                    nc.vector.memset(b8_fix[:], 0.25)

                for chunk in range(n_chunk):
                    csl = slice(chunk * (ch_idx // 16), (chunk + 1) * (ch_idx // 16))
                    if gathers:
                        a8 = pool.tile([P, cols, BATCH], FP8, tag="a8",
                                       bufs=ab_bufs)
                        b8 = pool.tile([P, cols, BATCH], FP8, tag="b8",
                                       bufs=ab_bufs)
                        nc.gpsimd.dma_gather(
                            out_ap=a8[:], in_ap=xt[:], idxs_ap=ia_all[:, csl],
                            num_idxs=ch_idx, num_idxs_reg=ch_idx, elem_size=BATCH,
                            queue_num=(2 * chunk) % queues)
                        nc.gpsimd.dma_gather(
                            out_ap=b8[:], in_ap=xt[:], idxs_ap=ib_all[:, csl],
                            num_idxs=ch_idx, num_idxs_reg=ch_idx, elem_size=BATCH,
                            queue_num=(2 * chunk + 1) % queues)
                    else:
                        a8, b8 = a8_fix, b8_fix
                    if not compute:
                        if out_dma and gathers:
                            nc.sync.dma_start(
                                out=out[:, chunk * cols:(chunk + 1) * cols, :256],
                                in_=a8[:].bitcast(FP16))
                        continue

                    a16 = pool.tile([P, cols, BATCH], FP16, tag="a16")
                    b16 = pool.tile([P, cols, BATCH], FP16, tag="b16")
                    t1 = pool.tile([P, cols, BATCH], FP16, tag="t1")
                    t2 = pool.tile([P, cols, BATCH], FP16, tag="t2")
                    out_t = pool.tile([P, cols, BATCH], FP16, tag="out_t")
                    for q in range(cols // 2):
                        qs = slice(2 * q, 2 * q + 2)
                        psa = ppool.tile([P, 2, BATCH], F32, tag="psa")
                        psb = ppool.tile([P, 2, BATCH], F32, tag="psb")
                        for j in range(2):
                            lc = 2 * q + j
                            nc.tensor.matmul(psa[:, j, :], ident_sb[:],
                                             a8[:, lc, :], start=True, stop=True)
                            nc.tensor.matmul(psb[:, j, :], ident_sb[:],
                                             b8[:, lc, :], start=True, stop=True)
                        nc.scalar.activation(a16[:, qs, :], psa[:],
                                             mybir.ActivationFunctionType.Copy)
                        nc.scalar.activation(b16[:, qs, :], psb[:],
                                             mybir.ActivationFunctionType.Copy)
                        for j in range(2):
                            lc = 2 * q + j
                            col = chunk * cols + lc
                            nc.vector.tensor_scalar(
                                out=t1[:, lc, :], in0=a16[:, lc, :],
                                scalar1=k[3][:, col:col + 1],
                                scalar2=k[2][:, col:col + 1],
                                op0=mybir.AluOpType.mult,
                                op1=mybir.AluOpType.add)
                            if lc < t2_act_cols:
                                nc.scalar.activation(
                                    t2[:, lc, :], a16[:, lc, :],
                                    mybir.ActivationFunctionType.Identity,
                                    scale=k[1][:, col:col + 1],
                                    bias=k[0][:, col:col + 1])
                            else:
                                nc.vector.tensor_scalar(
                                    out=t2[:, lc, :], in0=a16[:, lc, :],
                                    scalar1=k[1][:, col:col + 1],
                                    scalar2=k[0][:, col:col + 1],
                                    op0=mybir.AluOpType.mult,
                                    op1=mybir.AluOpType.add)
                        nc.vector.tensor_tensor(
                            out=t1[:, qs, :], in0=t1[:, qs, :],
                            in1=b16[:, qs, :], op=mybir.AluOpType.mult)
                        nc.vector.tensor_tensor(
                            out=out_t[:, qs, :], in0=t1[:, qs, :],
                            in1=t2[:, qs, :], op=mybir.AluOpType.add)
                        if out_dma and store_halves and q % 2 == 1:
                            hs = slice(2 * q - 2, 2 * q + 2)
                            c0 = chunk * cols + 2 * q - 2
                            nc.sync.dma_start(out=out[:, c0:c0 + 4, :],
                                              in_=out_t[:, hs, :])
                    if out_dma and not store_halves:
                        nc.sync.dma_start(
                            out=out[:, chunk * cols:(chunk + 1) * cols, :],
                            in_=out_t[:])
    nc.compile()
    return nc


_CACHED_NC = None


def kernel(x, weights, connections):
    global _CACHED_NC
    if _CACHED_NC is None:
        _CACHED_NC = build_program_f2k(t2_act_cols=6)
    in_maps = make_in_maps_fp8(np.asarray(x), np.asarray(weights),
                               np.asarray(connections))
    last_err = None
    for _attempt in range(3):
        try:
            res = run_bass_kernel_spmd(_CACHED_NC, in_maps, list(range(N_CORES)))
            return assemble_output(res.results)
        except Exception as e:
            last_err = e
    raise last_err

